# revision 1
# baseline (speedup 1.0000x reference)
"""Trainium2 Bass kernel for nn_DepthCueRectification_Sp.

Data-parallel over batch: 8 batch elements -> 8 NeuronCores (SPMD, one
program; per-core input maps carry that core's batch slice).

Per-core pipeline (D=768, N=1024, padded token dim NPAD=1152):
  tT    = U @ xb.T                     (f32r matmuls)
  xs_kT = (|S_k|*U) contraction        (f32r)
  logits_k = xs_k @ yb.T               (f32r); softmax rows on ACT/DVE
  pos   = softmax(-|p_temp| * sum_c coords[:,:,c]*pos_emb[:,c]) (fused DVE)
  attn_k = (1-g)*patch_k + g*pos ; entropy -> routing -> heat map
  attn_c -> PE transpose -> y_outT (bf16 matmuls)
  MLP: hT = W1 contraction of concat(x,y_full).T (bf16) -> fused bias+gelu
       xp = geluT contraction with W2 (bf16)
  out   = x + heat*(xp + b2)
"""

import sys

if "/opt/trn_rl_repo" not in sys.path:
    sys.path.insert(0, "/opt/trn_rl_repo")

import numpy as np
import ml_dtypes

import concourse.bass as bass
import concourse.mybir as mybir
import concourse.tile as tile
from concourse import bacc
from concourse.bass_utils import run_bass_kernel_spmd
from concourse.masks import make_identity

B, N, D, DFF, CLS = 8, 1024, 768, 3072, 1
NP1 = N + CLS          # 1025
NPAD = 1152            # 9 * 128; padded token dim for clean 512/512/128 chunks
ND = D // 128          # 6
NE = D // 128          # 6
NB = N // 128          # 8
NF = DFF // 128        # 24
NC2 = 2 * D // 128     # 12
AF = mybir.ActivationFunctionType
ALU = mybir.AluOpType
dt = mybir.dt

_prog_cache = {}


def _build(g, ht, pt):
    """Build + compile the SPMD program with baked scalars."""
    scale = float(D) ** -0.5
    omg = 1.0 - g

    nc = bacc.Bacc("TRN2", target_bir_lowering=False, debug=False, num_devices=8)

    # ---- DRAM params ----
    xt_d = nc.declare_dram_parameter("xt", [D, NP1], dt.float32r, isOutput=False)
    yt_d = nc.declare_dram_parameter("yt", [D, NP1], dt.float32r, isOutput=False)
    xtb_d = nc.declare_dram_parameter("xtb", [D, NPAD], dt.bfloat16, isOutput=False)
    ybn_d = nc.declare_dram_parameter("ybn", [N, D], dt.bfloat16, isOutput=False)
    utt_d = nc.declare_dram_parameter("utt", [ND, ND, 128, 128], dt.float32r, isOutput=False)
    us1_d = nc.declare_dram_parameter("us1", [NE, ND, 128, 128], dt.float32r, isOutput=False)
    us2_d = nc.declare_dram_parameter("us2", [NE, ND, 128, 128], dt.float32r, isOutput=False)
    w1t_d = nc.declare_dram_parameter("w1t", [NC2, NF, 128, 128], dt.bfloat16, isOutput=False)
    w2r_d = nc.declare_dram_parameter("w2r", [DFF, D], dt.bfloat16, isOutput=False)
    b1_d = nc.declare_dram_parameter("b1c", [DFF, 1], dt.float32, isOutput=False)
    b2b_d = nc.declare_dram_parameter("b2b", [128, D], dt.float32, isOutput=False)
    cpl_d = nc.declare_dram_parameter("cpl", [6, N, N], dt.float32, isOutput=False)
    pe_d = nc.declare_dram_parameter("pem", [N, 6], dt.float32, isOutput=False)
    xnat_d = nc.declare_dram_parameter("xnat", [NPAD, D], dt.float32, isOutput=False)
    out_d = nc.declare_dram_parameter("out", [NPAD, D], dt.float32, isOutput=True)
    hmbuf = nc.dram_tensor("hmbuf", [NPAD, 1], dt.float32)

    with tile.TileContext(nc) as tc:
        with tc.tile_pool(name="p0", bufs=1) as P0:
            # long-lived tiles
            xtb = [P0.tile([128, NPAD], dt.bfloat16, tag=f"xtb{d}", name=f"xtb{d}") for d in range(ND)]
            ybn = [P0.tile([128, D], dt.bfloat16, tag=f"ybn{m}", name=f"ybn{m}") for m in range(NB)]
            yfT = [P0.tile([128, NPAD], dt.bfloat16, tag=f"yfT{d}", name=f"yfT{d}") for d in range(ND)]
            b2b = P0.tile([128, D], dt.float32, tag="b2b", name="b2b")
            ident = P0.tile([128, 128], dt.float32, tag="ident", name="ident")
            onep = P0.tile([1, 1], dt.float32, tag="onep", name="onep")
            zerop = P0.tile([128, 1], dt.float32, tag="zerop", name="zerop")
            epsb = P0.tile([128, 1], dt.float32, tag="epsb", name="epsb")

            make_identity(nc, ident[:])
            nc.gpsimd.memset(onep[:], 1.0)
            nc.gpsimd.memset(zerop[:], 0.0)
            nc.gpsimd.memset(epsb[:], 1e-8)
            # heat buffer endpoints: hmbuf[0]=1 (CLS), pad rows [1025:] = 0
            nc.sync.dma_start(hmbuf[0:1, 0:1], onep[:])
            nc.sync.dma_start(hmbuf[NP1:NPAD, 0:1], zerop[0 : NPAD - NP1, 0:1])
            # pad cols of y_fullT zeroed once
            for d in range(ND):
                nc.gpsimd.memset(yfT[d][:, NP1:NPAD], 0.0)

            with tc.tile_pool(name="pyt", bufs=1) as PYT, \
                 tc.tile_pool(name="pxs", bufs=1) as PXS:
                yt = [PYT.tile([128, NP1], dt.float32r, tag=f"yt{e}", name=f"yt{e}") for e in range(NE)]
                xs = [[PXS.tile([128, N], dt.float32r, tag=f"xs{k}_{e}", name=f"xs{k}_{e}") for e in range(NE)]
                      for k in range(2)]

                # ---------- phase 1: tT then xs ----------
                with tc.tile_pool(name="p1", bufs=1) as P1, \
                     tc.tile_pool(name="p1s", bufs=12) as P1S, \
                     tc.tile_pool(name="ps1", bufs=2, space="PSUM") as PS1:
                    xt = [P1.tile([128, NP1], dt.float32r, tag=f"xt{d}", name=f"xt{d}") for d in range(ND)]
                    for d in range(ND):
                        nc.sync.dma_start(xt[d][:], xt_d[128 * d : 128 * d + 128, :])
                    for e in range(NE):
                        nc.sync.dma_start(yt[e][:], yt_d[128 * e : 128 * e + 128, :])
                    for d in range(ND):
                        nc.sync.dma_start(xtb[d][:], xtb_d[128 * d : 128 * d + 128, :])
                    for m in range(NB):
                        nc.sync.dma_start(ybn[m][:], ybn_d[128 * m : 128 * m + 128, :])
                    nc.sync.dma_start(b2b[:], b2b_d[:])
                    tT = [P1.tile([128, N], dt.float32r, tag=f"tT{d}", name=f"tT{d}") for d in range(ND)]

                    for d in range(ND):
                        utd = []
                        for k in range(ND):
                            w = P1S.tile([128, 128], dt.float32r, tag="wstream", name="wstream")
                            nc.sync.dma_start(w[:], utt_d[d, k])
                            utd.append(w)
                        ps = PS1.tile([128, N], dt.float32, tag="psA", name="psA")
                        for k in range(ND):
                            for h in range(2):
                                nc.tensor.matmul(
                                    ps[:, 512 * h : 512 * h + 512],
                                    utd[k][:],
                                    xt[k][:, CLS + 512 * h : CLS + 512 * h + 512],
                                    start=(k == 0), stop=(k == ND - 1),
                                )
                        nc.scalar.copy(tT[d][:], ps[:])

                    for k2 in range(2):
                        usk_d = us1_d if k2 == 0 else us2_d
                        for e in range(NE):
                            usd = []
                            for d in range(ND):
                                w = P1S.tile([128, 128], dt.float32r, tag="wstream", name="wstream")
                                nc.sync.dma_start(w[:], usk_d[e, d])
                                usd.append(w)
                            ps = PS1.tile([128, N], dt.float32, tag="psA", name="psA")
                            for d in range(ND):
                                for h in range(2):
                                    nc.tensor.matmul(
                                        ps[:, 512 * h : 512 * h + 512],
                                        usd[d][:],
                                        tT[d][:, 512 * h : 512 * h + 512],
                                        start=(d == 0), stop=(d == ND - 1),
                                    )
                            nc.scalar.copy(xs[k2][e][:], ps[:])

                # ---------- phases 2+3: attention, routing, y_out ----------
                with tc.tile_pool(name="pact", bufs=1) as PACT:
                    acT = [PACT.tile([128, N], dt.bfloat16, tag=f"acT{m}", name=f"acT{m}")
                           for m in range(NB)]

                    with tc.tile_pool(name="p2", bufs=2) as P2, \
                         tc.tile_pool(name="p2c", bufs=3) as P2C, \
                         tc.tile_pool(name="p2s", bufs=4) as SM, \
                         tc.tile_pool(name="ps2", bufs=2, space="PSUM") as PS2, \
                         tc.tile_pool(name="pst", bufs=2, space="PSUM") as PST:
                        for nb in range(NB):
                            r0, r1 = 128 * nb, 128 * nb + 128
                            # --- pos rows for this block ---
                            pet = SM.tile([128, 6], dt.float32, tag="pet", name="pet")
                            nc.sync.dma_start(pet[:], pe_d[r0:r1, :])
                            posg = P2.tile([128, N], dt.float32, tag="posg", name="posg")
                            for c in range(6):
                                pl = P2C.tile([128, N], dt.float32, tag="cpl", name="cpl")
                                nc.sync.dma_start(pl[:], cpl_d[c, r0:r1, :])
                                if c == 0:
                                    nc.vector.tensor_scalar_mul(posg[:], pl[:], pet[:, 0:1])
                                else:
                                    nc.vector.scalar_tensor_tensor(
                                        posg[:], pl[:], pet[:, c : c + 1], posg[:],
                                        ALU.mult, ALU.add)
                            mn = SM.tile([128, 1], dt.float32, tag="mn", name="mn")
                            nc.vector.tensor_reduce(mn[:], posg[:],
                                                    axis=mybir.AxisListType.X, op=ALU.min)
                            pbias = SM.tile([128, 1], dt.float32, tag="pbias", name="pbias")
                            nc.vector.tensor_scalar_mul(pbias[:], mn[:], pt)
                            psum_s = SM.tile([128, 1], dt.float32, tag="psum_s", name="psum_s")
                            nc.scalar.activation(posg[:], posg[:], AF.Exp,
                                                 bias=pbias[:], scale=-pt,
                                                 accum_out=psum_s[:])
                            prg = SM.tile([128, 1], dt.float32, tag="prg", name="prg")
                            nc.vector.reciprocal(prg[:], psum_s[:])
                            nc.vector.tensor_scalar_mul(prg[:], prg[:], g)
                            nc.vector.tensor_scalar_mul(posg[:], posg[:], prg[:])

                            # --- patch_k, attn_k, entropy_k ---
                            attn = []
                            ents = []
                            for k2 in range(2):
                                psl = PS2.tile([128, N], dt.float32, tag="psl", name="psl")
                                for e in range(NE):
                                    for h in range(2):
                                        nc.tensor.matmul(
                                            psl[:, 512 * h : 512 * h + 512],
                                            xs[k2][e][:, r0:r1],
                                            yt[e][:, CLS + 512 * h : CLS + 512 * h + 512],
                                            start=(e == 0), stop=(e == NE - 1),
                                        )
                                mx = SM.tile([128, 1], dt.float32, tag="mx", name="mx")
                                nc.vector.tensor_reduce(mx[:], psl[:],
                                                        axis=mybir.AxisListType.X,
                                                        op=ALU.max)
                                nbias = SM.tile([128, 1], dt.float32, tag="nbias", name="nbias")
                                nc.vector.tensor_scalar_mul(nbias[:], mx[:], -scale)
                                esum = SM.tile([128, 1], dt.float32, tag="esum", name="esum")
                                patch = P2.tile([128, N], dt.float32, tag="patch", name="patch")
                                nc.scalar.activation(patch[:], psl[:], AF.Exp,
                                                     bias=nbias[:], scale=scale,
                                                     accum_out=esum[:])
                                rk = SM.tile([128, 1], dt.float32, tag="rk", name="rk")
                                nc.vector.reciprocal(rk[:], esum[:])
                                nc.vector.tensor_scalar_mul(rk[:], rk[:], omg)
                                # attn = patch*rk + posg   (in place over patch)
                                nc.vector.scalar_tensor_tensor(
                                    patch[:], patch[:], rk[:], posg[:],
                                    ALU.mult, ALU.add)
                                lnk = P2.tile([128, N], dt.float32, tag="lnk", name="lnk")
                                nc.scalar.activation(lnk[:], patch[:], AF.Ln, bias=epsb[:])
                                ent = SM.tile([128, 1], dt.float32, tag="ent", name="ent")
                                # ent <- sum(attn*ln) = -entropy; sign folded into
                                # the sigmoid scale below
                                nc.vector.tensor_mul(lnk[:], patch[:], lnk[:])
                                nc.vector.tensor_reduce(ent[:], lnk[:],
                                                        axis=mybir.AxisListType.X,
                                                        op=ALU.add)
                                attn.append(patch)
                                ents.append(ent)

                            # --- routing ---
                            hk = []
                            for k2 in range(2):
                                sg = SM.tile([128, 1], dt.float32, tag="sg", name="sg")
                                nc.scalar.activation(sg[:], ents[k2][:], AF.Sigmoid,
                                                     scale=-ht)
                                hv = SM.tile([128, 1], dt.float32, tag="hv", name="hv")
                                nc.vector.tensor_scalar(hv[:], sg[:], -2.0, 2.0,
                                                        ALU.mult, ALU.add)
                                hk.append(hv)
                            rsel = SM.tile([128, 1], dt.float32, tag="rsel", name="rsel")
                            nc.vector.tensor_tensor(rsel[:], hk[0][:], hk[1][:], ALU.is_ge)
                            hd = SM.tile([128, 1], dt.float32, tag="hd", name="hd")
                            nc.vector.tensor_sub(hd[:], hk[0][:], hk[1][:])
                            heat = SM.tile([128, 1], dt.float32, tag="heat", name="heat")
                            nc.vector.scalar_tensor_tensor(
                                heat[:], hd[:], rsel[:], hk[1][:], ALU.mult, ALU.add)
                            nc.sync.dma_start(hmbuf[CLS + r0 : CLS + r1, 0:1], heat[:])
                            # attn_c = attn1 + rsel*(attn0-attn1), built in dka
                            dka = P2.tile([128, N], dt.float32, tag="dka", name="dka")
                            nc.vector.tensor_sub(dka[:], attn[0][:], attn[1][:])
                            nc.vector.scalar_tensor_tensor(
                                dka[:], dka[:], rsel[:], attn[1][:], ALU.mult, ALU.add)
                            # transpose into acT (bf16)
                            for mb in range(NB):
                                pt_ps = PST.tile([128, 128], dt.float32, tag="pst", name="pst")
                                nc.tensor.transpose(
                                    pt_ps[:], dka[:, 128 * mb : 128 * mb + 128], ident[:])
                                nc.scalar.copy(acT[mb][:, r0:r1], pt_ps[:])

                    # ---------- phase 3: y_outT ----------
                    with tc.tile_pool(name="ps3", bufs=2, space="PSUM") as PS3:
                        for d in range(ND):
                            psy = PS3.tile([128, N], dt.float32, tag="psy", name="psy")
                            for h in range(2):
                                for mb in range(NB):
                                    nc.tensor.matmul(
                                        psy[:, 512 * h : 512 * h + 512],
                                        ybn[mb][:, 128 * d : 128 * d + 128],
                                        acT[mb][:, 512 * h : 512 * h + 512],
                                        start=(mb == 0), stop=(mb == NB - 1),
                                    )
                            nc.scalar.copy(yfT[d][:, CLS : CLS + N], psy[:])
                            nc.scalar.copy(yfT[d][:, 0:CLS], yt[d][:, 0:CLS])

            # ---------- phase 4: MLP ----------
            with tc.tile_pool(name="pg", bufs=1) as PG, \
                 tc.tile_pool(name="pw", bufs=24) as PW:
                gel = [PG.tile([128, NPAD], dt.bfloat16, tag=f"gel{f}", name=f"gel{f}") for f in range(NF)]
                w2r = [PG.tile([128, D], dt.bfloat16, tag=f"w2r{f}", name=f"w2r{f}") for f in range(NF)]
                for f in range(NF):
                    nc.sync.dma_start(w2r[f][:], w2r_d[128 * f : 128 * f + 128, :])

                chunks1 = [(0, 512), (512, 512), (1024, NPAD - 1024)]
                with tc.tile_pool(name="ps4", bufs=2, space="PSUM") as PS4:
                    for f in range(NF):
                        psh = PS4.tile([128, NPAD], dt.float32, tag="psh", name="psh")
                        for c in range(NC2):
                            w = PW.tile([128, 128], dt.bfloat16, tag="w1s", name="w1s")
                            nc.sync.dma_start(w[:], w1t_d[c, f])
                            rhs = xtb[c] if c < ND else yfT[c - ND]
                            for (s0, wd) in chunks1:
                                nc.tensor.matmul(
                                    psh[:, s0 : s0 + wd], w[:], rhs[:, s0 : s0 + wd],
                                    start=(c == 0), stop=(c == NC2 - 1),
                                )
                        b1f = PW.tile([128, 1], dt.float32, tag="b1f", name="b1f")
                        nc.sync.dma_start(b1f[:], b1_d[128 * f : 128 * f + 128, 0:1])
                        nc.scalar.activation(gel[f][:], psh[:], AF.Gelu, bias=b1f[:])

                with tc.tile_pool(name="p5", bufs=3) as P5, \
                     tc.tile_pool(name="ps5", bufs=2, space="PSUM") as PS5:
                    chunks2 = [(0, 512), (512, D - 512)]
                    for nb in range(NPAD // 128):
                        r0 = 128 * nb
                        nrows = min(128, NP1 - r0)
                        if nrows <= 0:
                            continue
                        pso = PS5.tile([128, D], dt.float32, tag="pso", name="pso")
                        for f in range(NF):
                            for (s0, wd) in chunks2:
                                nc.tensor.matmul(
                                    pso[:, s0 : s0 + wd],
                                    gel[f][:, r0 : r0 + 128],
                                    w2r[f][:, s0 : s0 + wd],
                                    start=(f == 0), stop=(f == NF - 1),
                                )
                        st = P5.tile([128, D], dt.float32, tag="fin1", name="fin1")
                        nc.vector.tensor_add(st[:], pso[:], b2b[:])
                        xn = P5.tile([128, D], dt.float32, tag="xn", name="xn")
                        nc.sync.dma_start(xn[:nrows, :], xnat_d[r0 : r0 + nrows, :])
                        hmc = P5.tile([128, 1], dt.float32, tag="hmc", name="hmc")
                        nc.sync.dma_start(hmc[:nrows, :], hmbuf[r0 : r0 + nrows, 0:1])
                        ot = P5.tile([128, D], dt.float32, tag="ot", name="ot")
                        nc.vector.scalar_tensor_tensor(
                            ot[:nrows, :], st[:nrows, :], hmc[:nrows, :],
                            xn[:nrows, :], ALU.mult, ALU.add)
                        nc.sync.dma_start(out_d[r0 : r0 + nrows, :], ot[:nrows, :])

    nc.compile()
    return nc


def _get_prog(g, ht, pt):
    key = (round(float(g), 9), round(float(ht), 9), round(float(pt), 9))
    if key not in _prog_cache:
        _prog_cache[key] = _build(*key)
    return _prog_cache[key]


def kernel(x, y, coords, U, S1, S2, gating, h_temp, p_temp, pos_emb, W1, b1, W2, b2):
    x = np.asarray(x, dtype=np.float32)
    y = np.asarray(y, dtype=np.float32)
    coords = np.asarray(coords, dtype=np.float32)
    U = np.asarray(U, dtype=np.float32)
    bf16 = ml_dtypes.bfloat16

    g = float(1.0 / (1.0 + np.exp(-float(np.asarray(gating)))))
    ht = float(np.asarray(h_temp))
    pt = abs(float(np.asarray(p_temp)))
    nc = _get_prog(g, ht, pt)

    # ---- shared (replicated) host prep ----
    UT = np.ascontiguousarray(U.T)
    utt = np.empty((ND, ND, 128, 128), np.float32)
    for d in range(ND):
        for k in range(ND):
            utt[d, k] = UT[128 * k : 128 * k + 128, 128 * d : 128 * d + 128]
    us = []
    for S in (S1, S2):
        UsK = np.abs(np.asarray(S, np.float32))[:, None] * U
        u_t = np.empty((NE, ND, 128, 128), np.float32)
        for e in range(NE):
            for d in range(ND):
                u_t[e, d] = UsK[128 * d : 128 * d + 128, 128 * e : 128 * e + 128]
        us.append(u_t)
    W1 = np.asarray(W1, np.float32)
    w1t = np.empty((NC2, NF, 128, 128), bf16)
    for c in range(NC2):
        for f in range(NF):
            w1t[c, f] = W1[128 * c : 128 * c + 128, 128 * f : 128 * f + 128].astype(bf16)
    w2r = np.asarray(W2, np.float32).astype(bf16)
    b1c = np.asarray(b1, np.float32).reshape(DFF, 1)
    b2b = np.broadcast_to(np.asarray(b2, np.float32), (128, D)).copy()
    cpl = np.ascontiguousarray(coords.transpose(2, 0, 1))
    pem = np.ascontiguousarray(np.asarray(pos_emb, np.float32)[:, :, 0])

    shared = {"utt": utt, "us1": us[0], "us2": us[1], "w1t": w1t, "w2r": w2r,
              "b1c": b1c, "b2b": b2b, "cpl": cpl, "pem": pem}

    in_maps = []
    for b in range(B):
        xtf = np.ascontiguousarray(x[b].T)            # [D, 1025]
        ytf = np.ascontiguousarray(y[b].T)
        xtbp = np.zeros((D, NPAD), bf16)
        xtbp[:, :NP1] = xtf.astype(bf16)
        xnat = np.zeros((NPAD, D), np.float32)
        xnat[:NP1] = x[b]
        m = dict(shared)
        m["xt"] = xtf
        m["yt"] = ytf
        m["xtb"] = xtbp
        m["ybn"] = y[b, CLS:, :].astype(bf16)
        m["xnat"] = xnat
        in_maps.append(m)

    res = run_bass_kernel_spmd(nc, in_maps, list(range(B)))
    out = np.stack([res.results[b]["out"][:NP1, :] for b in range(B)])
    return out.astype(np.float32)


if __name__ == "__main__":
    import time
    sys.path.insert(0, "/root/problem")
    from reference import setup_inputs, reference

    inp = {k: np.asarray(v) for k, v in setup_inputs().items()}
    t0 = time.time()
    got = kernel(**inp)
    print("kernel wall:", time.time() - t0)
    exp = np.asarray(reference(**inp))
    d = np.abs(got - exp)
    print("absmax_rel:", d.max() / np.abs(exp).max())
    print("rms_rel:", np.sqrt((d ** 2).mean()) / np.sqrt((exp ** 2).mean()))



# revision 23
# speedup vs baseline: 1.4744x; 1.4744x over previous
"""Trainium2 Bass kernel for nn_DepthCueRectification_Sp.

Data-parallel over batch: 8 batch elements -> 8 NeuronCores (SPMD).

Per-core pipeline (D=768, N=1024, token pad NPAD=1152):
  tT    = U @ xb.T                  (bf16)
  yUT   = U @ yb.T                  (bf16)   [algebra: logits_k =
                                     (|S_k|*t) @ (y@U.T).T]
  tsT_k = |S_k|-scaled copies of tT (ACT per-partition scale)
  pos   = exp(-|p|*sum_c coords*pe) (DVE chain, bf16; no max-sub)
  logits_k -> exp (no max-sub, fused row-sum) -> attn_k = 256*attn (bf16)
  entropy via Ln + fused mult-reduce; routing compared on raw accums;
  heat = 2e/(1+e), e = exp(-ht*H_sel)
  dka (selected attn) -> PE transpose -> acT fp8 pairs
  y_outT = fp8 DoubleRow (ybp pairs @ acT pairs) -> yf8 = 16*y_full.T fp8
  MLP W1: x-half bf16 (xtb @ 32*W1a) + y-half fp8 DR (yf8 @ 2*W1b) = 32*h
          CLS token's y-half is zeroed on device and supplied via the
          host-computed correction hct (exact), added before gelu.
  gel   = gelu(psh/32 + b1) (bf16)
  MLP W2: bf16 (gel @ W2)
  out   = x + heat*(xp + b2)
"""

import os
import sys

if "/opt/trn_rl_repo" not in sys.path:
    sys.path.insert(0, "/opt/trn_rl_repo")

import numpy as np
import ml_dtypes

import concourse.bass as bass
import concourse.mybir as mybir
import concourse.tile as tile
from concourse import bacc
from concourse.bass_utils import run_bass_kernel_spmd
from concourse.masks import make_identity

B, N, D, DFF, CLS = 8, 1024, 768, 3072, 1
NP1 = N + CLS          # 1025
NPAD = 1152            # 9 * 128
ND = D // 128          # 6
NB = N // 128          # 8
NF = DFF // 128        # 24
AF = mybir.ActivationFunctionType
ALU = mybir.AluOpType
dt = mybir.dt
DR = mybir.MatmulPerfMode.DoubleRow

NODR = bool(int(os.environ.get("K_NODR", "0")))    # disable DoubleRow matmuls
# tensor_tensor_reduce and 16-bit PE transposes both hard-crash the exec
# unit on this toolchain (NRT_EXEC_UNIT_UNRECOVERABLE) — keep them off.
NOTTR = bool(int(os.environ.get("K_NOTTR", "1")))
F32T = bool(int(os.environ.get("K_F32T", "1")))

SCALE = float(D) ** -0.5
SA = 256.0             # attn scale (fp8 headroom)
SY = 16.0              # y_full scale in yf8
SW1X = 32.0            # W1 x-half scale (bf16)  == SW1Y*SY
SW1Y = 2.0             # W1 y-half scale (fp8)
LN256 = float(np.log(SA))

_prog_cache = {}


def _build(g, ht, pt):
    omg = 1.0 - g
    f8 = dt.float8e4
    bf = dt.bfloat16
    f32 = dt.float32

    nc = bacc.Bacc("TRN2", target_bir_lowering=False, debug=False, num_devices=8)

    def mm_dr(out, l3, r3, start, stop):
        if not NODR:
            nc.tensor.matmul(out, l3, r3, start=start, stop=stop, perf_mode=DR)
        else:
            nc.tensor.matmul(out, l3[:, 0], r3[:, 0], start=start, stop=False)
            nc.tensor.matmul(out, l3[:, 1], r3[:, 1], start=False, stop=stop)

    # ---- DRAM params ----
    xtb_d = nc.declare_dram_parameter("xtb", [128, ND, NPAD], bf, isOutput=False)
    yt_d = nc.declare_dram_parameter("yt", [128, ND, NP1], bf, isOutput=False)
    ybp_d = nc.declare_dram_parameter("ybp", [128, 4, 2, D], f8, isOutput=False)
    utb_d = nc.declare_dram_parameter("utb", [128, ND, ND, 128], bf, isOutput=False)
    w1p_d = nc.declare_dram_parameter("w1p", [128, 3, NF, 2, 128], f8, isOutput=False)
    w1x_d = nc.declare_dram_parameter("w1x", [128, ND, NF, 128], bf, isOutput=False)
    w2r_d = nc.declare_dram_parameter("w2r", [128, NF, D], bf, isOutput=False)
    b1t_d = nc.declare_dram_parameter("b1t", [128, NF], f32, isOutput=False)
    hct_d = nc.declare_dram_parameter("hct", [128, NF], f32, isOutput=False)
    b2b_d = nc.declare_dram_parameter("b2b", [128, D], f32, isOutput=False)
    s12_d = nc.declare_dram_parameter("s12", [128, 2, ND], f32, isOutput=False)
    pem_d = nc.declare_dram_parameter("pem", [128, NB, 6], f32, isOutput=False)
    cpl_d = nc.declare_dram_parameter("cpl", [NB, 128, 6, N], bf, isOutput=False)
    xnat_d = nc.declare_dram_parameter("xnat", [NPAD, D], f32, isOutput=False)
    out_d = nc.declare_dram_parameter("out", [NPAD, D], f32, isOutput=True)
    hmbuf = nc.dram_tensor("hmbuf", [NPAD, 1], f32)

    with tile.TileContext(nc) as tc:
        with tc.tile_pool(name="p0", bufs=1) as P0:
            # ---- persistent tiles ----
            w1p = P0.tile([128, 3, NF, 2, 128], f8, tag="w1p", name="w1p")
            xtb = P0.tile([128, ND, NPAD], bf, tag="xtb", name="xtb")
            yf8 = P0.tile([128, 3, 2, NPAD], f8, tag="yf8", name="yf8")
            b2b = P0.tile([128, D], f32, tag="b2b", name="b2b")
            b1t = P0.tile([128, NF], f32, tag="b1t", name="b1t")
            hct = P0.tile([128, NF], f32, tag="hct", name="hct")
            s12 = P0.tile([128, 2, ND], f32, tag="s12", name="s12")
            pem = P0.tile([128, NB, 6], f32, tag="pem", name="pem")
            tdt = f32 if F32T else bf
            identb = P0.tile([128, 128], tdt, tag="identb", name="identb")
            epsb = P0.tile([128, 1], f32, tag="epsb", name="epsb")
            onep = P0.tile([1, 1], f32, tag="onep", name="onep")
            zerop = P0.tile([128, 1], f32, tag="zerop", name="zerop")
            hbias = P0.tile([128, 1], f32, tag="hbias", name="hbias")

            # ---- gpsimd queue: weights + inits ----
            nc.gpsimd.dma_start(s12[:], s12_d[:])
            nc.gpsimd.dma_start(pem[:], pem_d[:])
            nc.gpsimd.dma_start(w1p[:], w1p_d[:])
            make_identity(nc, identb[:])
            nc.gpsimd.memset(epsb[:], SA * 1e-8)
            nc.gpsimd.memset(hbias[:], -ht * LN256)
            nc.gpsimd.memset(onep[:], 1.0)
            nc.gpsimd.memset(zerop[:], 0.0)
            # yf8 pad cols + CLS col (CLS y-half comes from hct instead)
            nc.gpsimd.memset(yf8[:, :, :, NP1:NPAD], 0.0)
            nc.gpsimd.memset(yf8[:, :, :, 0:CLS], 0.0)
            nc.gpsimd.dma_start(hmbuf[0:1, 0:1], onep[:])
            nc.gpsimd.dma_start(hmbuf[NP1:NPAD, 0:1], zerop[0 : NPAD - NP1, 0:1])

            # ---- scalar queue: small consts ----
            nc.scalar.dma_start(b2b[:], b2b_d[:])
            nc.scalar.dma_start(b1t[:], b1t_d[:])
            nc.scalar.dma_start(hct[:], hct_d[:])

            with tc.tile_pool(name="pa2", bufs=1) as PA2:
                acT = PA2.tile([128, 4, 2, N], f8, tag="acT", name="acT")
                ybp = PA2.tile([128, 4, 2, D], f8, tag="ybp", name="ybp")

                with tc.tile_pool(name="pa1", bufs=1) as PA1:
                    yUT = PA1.tile([128, ND, N], bf, tag="yUT", name="yUT")
                    ts0 = PA1.tile([128, ND, N], bf, tag="ts0", name="ts0")
                    ts1 = PA1.tile([128, ND, N], bf, tag="ts1", name="ts1")
                    posn = PA1.tile([128, NB, N], bf, tag="posn", name="posn")

                    # ---------- phase 1: tT (scaled copies) and yUT ----------
                    with tc.tile_pool(name="p1", bufs=1) as P1, \
                         tc.tile_pool(name="ps1", bufs=2, space="PSUM") as PS1:
                        utb = P1.tile([128, ND, ND, 128], bf, tag="utb", name="utb")
                        nc.gpsimd.dma_start(utb[:], utb_d[:])
                        yt = P1.tile([128, ND, NP1], bf, tag="yt", name="yt")
                        nc.sync.dma_start(xtb[:], xtb_d[:])
                        nc.sync.dma_start(yt[:], yt_d[:])

                        for d in range(ND):
                            ps = PS1.tile([128, N], f32, tag="psA", name="psA")
                            for k in range(ND):
                                for h in range(2):
                                    nc.tensor.matmul(
                                        ps[:, 512 * h : 512 * h + 512],
                                        utb[:, d, k],
                                        xtb[:, k, CLS + 512 * h : CLS + 512 * h + 512],
                                        start=(k == 0), stop=(k == ND - 1),
                                    )
                            nc.scalar.mul(ts0[:, d, :], ps[:], s12[:, 0, d : d + 1])
                            nc.scalar.mul(ts1[:, d, :], ps[:], s12[:, 1, d : d + 1])
                        for d in range(ND):
                            ps = PS1.tile([128, N], f32, tag="psA", name="psA")
                            for k in range(ND):
                                for h in range(2):
                                    nc.tensor.matmul(
                                        ps[:, 512 * h : 512 * h + 512],
                                        utb[:, d, k],
                                        yt[:, k, CLS + 512 * h : CLS + 512 * h + 512],
                                        start=(k == 0), stop=(k == ND - 1),
                                    )
                            nc.scalar.copy(yUT[:, d, :], ps[:])

                        # ---------- pos: unnormalized exp, SA*g/rowsum folded ----------
                        with tc.tile_pool(name="pcp", bufs=2) as CPP, \
                             tc.tile_pool(name="ppo", bufs=2) as PO, \
                             tc.tile_pool(name="psm0", bufs=8) as SM0:
                            for nb in range(NB):
                                cpt = CPP.tile([128, 6, N], bf, tag="cpt", name="cpt")
                                nc.sync.dma_start(cpt[:], cpl_d[nb])
                                if nb == 2:
                                    nc.sync.dma_start(ybp[:], ybp_d[:])
                                pga = PO.tile([128, N], bf, tag="pga", name="pga")
                                nc.vector.tensor_scalar_mul(
                                    pga[:], cpt[:, 0], pem[:, nb, 0:1])
                                for c in range(1, 6):
                                    nc.vector.scalar_tensor_tensor(
                                        pga[:], cpt[:, c], pem[:, nb, c : c + 1],
                                        pga[:], ALU.mult, ALU.add)
                                pxp = PO.tile([128, N], bf, tag="pxp", name="pxp")
                                pss = SM0.tile([128, 1], f32, tag="pss", name="pss")
                                nc.scalar.activation(pxp[:], pga[:], AF.Exp,
                                                     bias=zerop[:],
                                                     accum_out=pss[:])
                                prg = SM0.tile([128, 1], f32, tag="prg", name="prg")
                                nc.vector.reciprocal(prg[:], pss[:])
                                nc.vector.tensor_scalar_mul(prg[:], prg[:], SA * g)
                                nc.vector.tensor_scalar_mul(
                                    posn[:, nb, :], pxp[:], prg[:])

                    # ---------- phase 2: attention, entropy, routing ----------
                    with tc.tile_pool(name="pat", bufs=4) as PT, \
                         tc.tile_pool(name="plk", bufs=2) as LK, \
                         tc.tile_pool(name="pdk", bufs=4) as DK, \
                         tc.tile_pool(name="psm", bufs=16) as SM, \
                         tc.tile_pool(name="psl", bufs=3, space="PSUM") as PSL, \
                         tc.tile_pool(name="pstp", bufs=2, space="PSUM") as PST:
                        for nb in range(NB):
                            r0 = 128 * nb
                            attn = []
                            accs = []
                            for k2 in range(2):
                                tsk = ts0 if k2 == 0 else ts1
                                psl = PSL.tile([128, N], f32, tag="psl", name="psl")
                                for e in range(ND):
                                    for h in range(2):
                                        nc.tensor.matmul(
                                            psl[:, 512 * h : 512 * h + 512],
                                            tsk[:, e, r0 : r0 + 128],
                                            yUT[:, e, 512 * h : 512 * h + 512],
                                            start=(e == 0), stop=(e == ND - 1),
                                        )
                                patch = PT.tile([128, N], bf, tag="patch", name="patch")
                                esum = SM.tile([128, 1], f32, tag="esum", name="esum")
                                nc.scalar.activation(patch[:], psl[:], AF.Exp,
                                                     bias=zerop[:], scale=SCALE,
                                                     accum_out=esum[:])
                                rk = SM.tile([128, 1], f32, tag="rk", name="rk")
                                nc.vector.reciprocal(rk[:], esum[:])
                                nc.vector.tensor_scalar_mul(rk[:], rk[:], SA * omg)
                                nc.vector.scalar_tensor_tensor(
                                    patch[:], patch[:], rk[:], posn[:, nb, :],
                                    ALU.mult, ALU.add)
                                lnk = LK.tile([128, N], bf, tag="lnk", name="lnk")
                                nc.scalar.activation(lnk[:], patch[:], AF.Ln,
                                                     bias=epsb[:])
                                acc = SM.tile([128, 1], f32, tag="acc", name="acc")
                                # acc = -ht/SA*sum(attn_s*ln attn_s) = ht*H - ht*lnSA
                                if NOTTR:
                                    nc.vector.tensor_mul(lnk[:], lnk[:], patch[:])
                                    nc.vector.tensor_reduce(
                                        acc[:], lnk[:], axis=mybir.AxisListType.X,
                                        op=ALU.add)
                                    nc.vector.tensor_scalar_mul(
                                        acc[:], acc[:], -ht / SA)
                                else:
                                    nc.vector.tensor_tensor_reduce(
                                        lnk[:], lnk[:], patch[:], -ht / SA, 0.0,
                                        ALU.mult, ALU.add, accum_out=acc[:])
                                attn.append(patch)
                                accs.append(acc)

                            rsel = SM.tile([128, 1], f32, tag="rsel", name="rsel")
                            nc.vector.tensor_tensor(rsel[:], accs[1][:], accs[0][:],
                                                    ALU.is_ge)
                            amin = SM.tile([128, 1], f32, tag="amin", name="amin")
                            nc.vector.tensor_tensor(amin[:], accs[0][:], accs[1][:],
                                                    ALU.min)
                            ee = SM.tile([128, 1], f32, tag="ee", name="ee")
                            nc.scalar.activation(ee[:], amin[:], AF.Exp,
                                                 scale=-1.0, bias=hbias[:])
                            ep1 = SM.tile([128, 1], f32, tag="ep1", name="ep1")
                            nc.vector.tensor_scalar_add(ep1[:], ee[:], 1.0)
                            rcp = SM.tile([128, 1], f32, tag="rcp", name="rcp")
                            nc.vector.reciprocal(rcp[:], ep1[:])
                            heat = SM.tile([128, 1], f32, tag="heat", name="heat")
                            nc.vector.scalar_tensor_tensor(
                                heat[:], ee[:], 2.0, rcp[:], ALU.mult, ALU.mult)
                            nc.sync.dma_start(
                                hmbuf[CLS + r0 : CLS + r0 + 128, 0:1], heat[:])
                            d01 = DK.tile([128, N], bf, tag="d01", name="d01")
                            nc.vector.tensor_sub(d01[:], attn[0][:], attn[1][:])
                            dka = DK.tile([128, N], tdt, tag="dka", name="dka")
                            nc.vector.scalar_tensor_tensor(
                                dka[:], d01[:], rsel[:], attn[1][:],
                                ALU.mult, ALU.add)
                            for mb in range(NB):
                                pst = PST.tile([128, 128], tdt, tag="pst", name="pst")
                                nc.tensor.transpose(
                                    pst[:], dka[:, 128 * mb : 128 * mb + 128],
                                    identb[:])
                                dst = acT[:, mb // 2, mb % 2, r0 : r0 + 128]
                                if mb < 4:
                                    nc.scalar.copy(dst, pst[:])
                                else:
                                    nc.vector.tensor_copy(dst, pst[:])

                # ---------- phase 3: y_outT (fp8 DoubleRow) -> yf8 ----------
                with tc.tile_pool(name="psy", bufs=2, space="PSUM") as PSY:
                    for d in range(ND):
                        psy = PSY.tile([128, N], f32, tag="psy", name="psy")
                        for mbp in range(4):
                            for h in range(2):
                                mm_dr(
                                    psy[:, 512 * h : 512 * h + 512],
                                    ybp[:, mbp, :, 128 * d : 128 * d + 128],
                                    acT[:, mbp, :, 512 * h : 512 * h + 512],
                                    (mbp == 0), (mbp == 3),
                                )
                        nc.scalar.mul(yf8[:, d // 2, d % 2, CLS : CLS + N],
                                      psy[:], SY / SA)

            # ---------- phase 4: MLP ----------
            with tc.tile_pool(name="pg", bufs=1) as PG:
                w1x = PG.tile([128, ND, NF, 128], bf, tag="w1x", name="w1x")
                w2r = PG.tile([128, NF, D], bf, tag="w2r", name="w2r")
                gel = PG.tile([128, NF, NPAD], bf, tag="gel", name="gel")
                nc.gpsimd.dma_start(w1x[:], w1x_d[:])
                nc.sync.dma_start(w2r[:], w2r_d[:])

                chunksA = [(1024, NPAD - 1024), (0, 512), (512, 512)]
                with tc.tile_pool(name="psh", bufs=2, space="PSUM") as PSH:
                    for f in range(NF):
                        psh = PSH.tile([128, NPAD], f32, tag="psh", name="psh")
                        for c in range(ND):
                            for (s0, wd) in chunksA:
                                nc.tensor.matmul(
                                    psh[:, s0 : s0 + wd],
                                    w1x[:, c, f],
                                    xtb[:, c, s0 : s0 + wd],
                                    start=(c == 0), stop=False,
                                )
                        for yp in range(3):
                            for (s0, wd) in chunksA:
                                mm_dr(
                                    psh[:, s0 : s0 + wd],
                                    w1p[:, yp, f],
                                    yf8[:, yp, :, s0 : s0 + wd],
                                    False, (yp == 2),
                                )
                        # exact CLS-token y-half correction (host-computed)
                        nc.vector.tensor_add(psh[:, 0:CLS], psh[:, 0:CLS],
                                             hct[:, f : f + 1])
                        nc.scalar.activation(gel[:, f, :], psh[:],
                                             AF.Gelu, bias=b1t[:, f : f + 1],
                                             scale=1.0 / SW1X)

                with tc.tile_pool(name="p5", bufs=3) as P5, \
                     tc.tile_pool(name="pso", bufs=2, space="PSUM") as PSO:
                    chunksB = [(512, D - 512), (0, 512)]
                    for tb in range(NPAD // 128):
                        r0 = 128 * tb
                        nrows = min(128, NP1 - r0)
                        if nrows <= 0:
                            continue
                        xn = P5.tile([128, D], f32, tag="xn", name="xn")
                        nc.sync.dma_start(xn[:nrows, :], xnat_d[r0 : r0 + nrows, :])
                        hmc = P5.tile([128, 1], f32, tag="hmc", name="hmc")
                        nc.sync.dma_start(hmc[:nrows, :],
                                          hmbuf[r0 : r0 + nrows, 0:1])
                        pso = PSO.tile([128, D], f32, tag="pso", name="pso")
                        for f in range(NF):
                            for (s0, wd) in chunksB:
                                nc.tensor.matmul(
                                    pso[:, s0 : s0 + wd],
                                    gel[:, f, r0 : r0 + 128],
                                    w2r[:, f, s0 : s0 + wd],
                                    start=(f == 0), stop=(f == NF - 1),
                                )
                        st = P5.tile([128, D], f32, tag="st", name="st")
                        nc.vector.tensor_add(st[:], pso[:], b2b[:])
                        ot = P5.tile([128, D], f32, tag="ot", name="ot")
                        nc.vector.scalar_tensor_tensor(
                            ot[:nrows, :], st[:nrows, :], hmc[:nrows, :],
                            xn[:nrows, :], ALU.mult, ALU.add)
                        nc.sync.dma_start(out_d[r0 : r0 + nrows, :], ot[:nrows, :])

    nc.compile()
    return nc


def _get_prog(g, ht, pt):
    key = (round(float(g), 9), round(float(ht), 9), round(float(pt), 9))
    if key not in _prog_cache:
        _prog_cache[key] = _build(*key)
    return _prog_cache[key]


def kernel(x, y, coords, U, S1, S2, gating, h_temp, p_temp, pos_emb, W1, b1, W2, b2):
    x = np.asarray(x, dtype=np.float32)
    y = np.asarray(y, dtype=np.float32)
    coords = np.asarray(coords, dtype=np.float32)
    U = np.asarray(U, dtype=np.float32)
    bf16 = ml_dtypes.bfloat16
    f8 = ml_dtypes.float8_e4m3

    g = float(1.0 / (1.0 + np.exp(-float(np.asarray(gating)))))
    ht = float(np.asarray(h_temp))
    pt = abs(float(np.asarray(p_temp)))
    nc = _get_prog(g, ht, pt)

    def q8(a):
        return np.clip(a, -240.0, 240.0).astype(f8)

    # ---- shared (replicated) host prep ----
    UT = np.ascontiguousarray(U.T)
    utb = np.ascontiguousarray(
        UT.reshape(ND, 128, ND, 128).transpose(1, 2, 0, 3)).astype(bf16)
    s12 = np.ascontiguousarray(np.stack(
        [np.abs(np.asarray(S1, np.float32)).reshape(ND, 128).T,
         np.abs(np.asarray(S2, np.float32)).reshape(ND, 128).T], axis=1))
    pem = np.ascontiguousarray(
        (-pt) * np.asarray(pos_emb, np.float32)[:, :, 0]
        .reshape(NB, 128, 6).transpose(1, 0, 2))
    cpl = np.ascontiguousarray(
        coords.reshape(NB, 128, N, 6).transpose(0, 1, 3, 2)).astype(bf16)
    W1 = np.asarray(W1, np.float32)
    W1a, W1b = W1[:D], W1[D:]
    w1x = np.ascontiguousarray(
        (SW1X * W1a).reshape(ND, 128, NF, 128).transpose(1, 0, 2, 3)).astype(bf16)
    w1p = q8((SW1Y * W1b).reshape(3, 2, 128, NF, 128).transpose(2, 0, 3, 1, 4))
    W2 = np.asarray(W2, np.float32)
    w2r = np.ascontiguousarray(
        W2.reshape(NF, 128, D).transpose(1, 0, 2)).astype(bf16)
    b1t = np.ascontiguousarray(np.asarray(b1, np.float32).reshape(NF, 128).T)
    b2b = np.broadcast_to(np.asarray(b2, np.float32), (128, D)).copy()

    shared = {"utb": utb, "s12": s12, "pem": pem, "cpl": cpl,
              "w1x": w1x, "w1p": w1p, "w2r": w2r, "b1t": b1t, "b2b": b2b}

    in_maps = []
    for b in range(B):
        xp = np.zeros((NPAD, D), np.float32)
        xp[:NP1] = x[b]
        xtb = np.ascontiguousarray(
            xp.T.reshape(ND, 128, NPAD).transpose(1, 0, 2)).astype(bf16)
        yt = np.ascontiguousarray(
            y[b].T.reshape(ND, 128, NP1).transpose(1, 0, 2)).astype(bf16)
        ybp = q8(y[b, CLS:].reshape(4, 2, 128, D).transpose(2, 0, 1, 3))
        xnat = np.zeros((NPAD, D), np.float32)
        xnat[:NP1] = x[b]
        # exact CLS y-half: 32*h_y[cls] = SW1X * (y_cls @ W1b)
        hc = SW1X * (y[b, 0] @ W1b)
        hct = np.ascontiguousarray(hc.reshape(NF, 128).T)
        m = dict(shared)
        m["xtb"] = xtb
        m["yt"] = yt
        m["ybp"] = ybp
        m["xnat"] = xnat
        m["hct"] = hct
        in_maps.append(m)

    res = run_bass_kernel_spmd(nc, in_maps, list(range(B)))
    out = np.stack([res.results[b]["out"][:NP1, :] for b in range(B)])
    return out.astype(np.float32)


if __name__ == "__main__":
    import time
    sys.path.insert(0, "/root/problem")
    from reference import setup_inputs, reference

    inp = {k: np.asarray(v) for k, v in setup_inputs().items()}
    t0 = time.time()
    got = kernel(**inp)
    print("kernel wall:", time.time() - t0)
    exp = np.asarray(reference(**inp))
    d = np.abs(got - exp)
    print("absmax_rel:", d.max() / np.abs(exp).max())
    print("rms_rel:", np.sqrt((d ** 2).mean()) / np.sqrt((exp ** 2).mean()))


# revision 25
# speedup vs baseline: 1.7257x; 1.1705x over previous
"""Trainium2 Bass kernel for nn_DepthCueRectification_Sp.

Data-parallel over batch: 8 batch elements -> 8 NeuronCores (SPMD).

Per-core pipeline (D=768, N=1024, token pad NPAD=1152):
  tT    = U @ xb.T                  (bf16)
  yUT   = U @ yb.T                  (bf16)   [algebra: logits_k =
                                     (|S_k|*t) @ (y@U.T).T]
  tsT_k = |S_k|-scaled copies of tT (ACT per-partition scale)
  pos   = PE identity-accumulate of host-prescaled coord planes
          (cplw[i,j,c] = -|p|*pe[i,c]*coords[i,j,c]), exp on ACT
  logits_k -> exp (no max-sub, fused row-sum) -> attn_k = 256*attn (bf16)
  entropy: Ln on ACT; multiply+reduce on Pool (raw accum); routing
  compares raw accums; heat = 2e/(1+e), e = exp(-ht*H_sel) via one exp
  dka (selected attn, f32) -> PE transpose (f32) -> acT fp8 pairs
  y_outT = fp8 DoubleRow (ybp pairs @ acT pairs) -> yf8 = 16*y_full.T fp8
  MLP W1: x-half bf16 (xtb @ 32*W1a) + y-half fp8 DR (yf8 @ 2*W1b) = 32*h
          CLS y-half zeroed on device; exact host correction hct added.
  gel   = gelu(psh/32 + b1) -> fp8 pairs
  MLP W2: fp8 DoubleRow (gel pairs @ 32*W2 pairs) = 32*xp'
  out   = x + heat*(xp' + b2)

The act-table dict is patched so Exp and Ln resolve to the combined
natural_log_exp_and_others set (avoids per-iteration table reloads).
tensor_tensor_reduce and 16-bit PE transposes hard-crash the exec unit
on this toolchain and are not used.
"""

import os
import sys

if "/opt/trn_rl_repo" not in sys.path:
    sys.path.insert(0, "/opt/trn_rl_repo")

import numpy as np
import ml_dtypes

import concourse.bass as bass
import concourse.mybir as mybir
import concourse.tile as tile
from concourse import bacc
from concourse.bass_utils import run_bass_kernel_spmd
from concourse.hw_specs import get_activation_tables
from concourse.masks import make_identity

B, N, D, DFF, CLS = 8, 1024, 768, 3072, 1
NP1 = N + CLS          # 1025
NPAD = 1152            # 9 * 128
ND = D // 128          # 6
NB = N // 128          # 8
NF = DFF // 128        # 24
AF = mybir.ActivationFunctionType
ALU = mybir.AluOpType
dt = mybir.dt
DR = mybir.MatmulPerfMode.DoubleRow

NODR = bool(int(os.environ.get("K_NODR", "0")))      # disable DoubleRow
NOPOOL = bool(int(os.environ.get("K_NOPOOL", "0")))  # entropy ops on DVE
W2BF = bool(int(os.environ.get("K_W2BF", "0")))      # W2 in bf16

SCALE = float(D) ** -0.5
SA = 256.0             # attn scale (fp8 headroom)
SY = 16.0              # y_full scale in yf8
SW1X = 32.0            # W1 x-half scale (bf16)  == SW1Y*SY
SW1Y = 2.0             # W1 y-half scale (fp8)
SW2 = 32.0             # W2 scale (fp8)
LN256 = float(np.log(SA))

_prog_cache = {}


def _patch_act_tables(arch):
    """Make natural_log_exp_and_others the only provider of Exp/Ln so the
    compiler's table-load pass keeps one table across the attention loop.
    Mutates the functools-cached dict in place (names/ids unchanged)."""
    tabs = get_activation_tables(arch)
    keep = "natural_log_exp_and_others"
    if keep not in tabs:
        return
    for name, s in tabs.items():
        if name == keep:
            continue
        s.discard(AF.Exp)
        s.discard(AF.Ln)


def _build(g, ht, pt):
    omg = 1.0 - g
    f8 = dt.float8e4
    bf = dt.bfloat16
    f32 = dt.float32

    nc = bacc.Bacc("TRN2", target_bir_lowering=False, debug=False, num_devices=8)
    _patch_act_tables(nc.m.arch)

    def mm_dr(out, l3, r3, start, stop):
        if not NODR:
            nc.tensor.matmul(out, l3, r3, start=start, stop=stop, perf_mode=DR)
        else:
            nc.tensor.matmul(out, l3[:, 0], r3[:, 0], start=start, stop=False)
            nc.tensor.matmul(out, l3[:, 1], r3[:, 1], start=False, stop=stop)

    ENT = nc.vector if NOPOOL else nc.gpsimd

    # ---- DRAM params ----
    xtb_d = nc.declare_dram_parameter("xtb", [128, ND, NPAD], bf, isOutput=False)
    yt_d = nc.declare_dram_parameter("yt", [128, ND, NP1], bf, isOutput=False)
    ybp_d = nc.declare_dram_parameter("ybp", [128, 4, 2, D], f8, isOutput=False)
    utb_d = nc.declare_dram_parameter("utb", [128, ND, ND, 128], bf, isOutput=False)
    w1p_d = nc.declare_dram_parameter("w1p", [128, 3, NF, 2, 128], f8, isOutput=False)
    w1x_d = nc.declare_dram_parameter("w1x", [128, ND, NF, 128], bf, isOutput=False)
    if W2BF:
        w2r_d = nc.declare_dram_parameter("w2r", [128, NF, D], bf, isOutput=False)
    else:
        w2r_d = nc.declare_dram_parameter("w2r", [128, 12, 2, D], f8, isOutput=False)
    b1t_d = nc.declare_dram_parameter("b1t", [128, NF], f32, isOutput=False)
    hct_d = nc.declare_dram_parameter("hct", [128, NF], f32, isOutput=False)
    b2b_d = nc.declare_dram_parameter("b2b", [128, D], f32, isOutput=False)
    s12_d = nc.declare_dram_parameter("s12", [128, 2, ND], f32, isOutput=False)
    cpl_d = nc.declare_dram_parameter("cpl", [NB, 128, 6, N], bf, isOutput=False)
    xnat_d = nc.declare_dram_parameter("xnat", [NPAD, D], f32, isOutput=False)
    out_d = nc.declare_dram_parameter("out", [NPAD, D], f32, isOutput=True)
    hmbuf = nc.dram_tensor("hmbuf", [NPAD, 1], f32)

    with tile.TileContext(nc) as tc:
        with tc.tile_pool(name="p0", bufs=1) as P0:
            # ---- persistent tiles ----
            w1p = P0.tile([128, 3, NF, 2, 128], f8, tag="w1p", name="w1p")
            w1x = P0.tile([128, ND, NF, 128], bf, tag="w1x", name="w1x")
            xtb = P0.tile([128, ND, NPAD], bf, tag="xtb", name="xtb")
            yf8 = P0.tile([128, 3, 2, NPAD], f8, tag="yf8", name="yf8")
            b2b = P0.tile([128, D], f32, tag="b2b", name="b2b")
            b1t = P0.tile([128, NF], f32, tag="b1t", name="b1t")
            hct = P0.tile([128, NF], f32, tag="hct", name="hct")
            s12 = P0.tile([128, 2, ND], f32, tag="s12", name="s12")
            identf = P0.tile([128, 128], f32, tag="identf", name="identf")
            identm = P0.tile([128, 128], bf, tag="identm", name="identm")
            epsb = P0.tile([128, 1], f32, tag="epsb", name="epsb")
            onep = P0.tile([1, 1], f32, tag="onep", name="onep")
            zerop = P0.tile([128, 1], f32, tag="zerop", name="zerop")
            hbias = P0.tile([128, 1], f32, tag="hbias", name="hbias")

            # ---- gpsimd queue: small inits, then the big weight loads ----
            nc.gpsimd.dma_start(s12[:], s12_d[:])
            make_identity(nc, identf[:])
            make_identity(nc, identm[:])
            nc.gpsimd.memset(epsb[:], SA * 1e-8)
            nc.gpsimd.memset(hbias[:], -ht * LN256)
            nc.gpsimd.memset(onep[:], 1.0)
            nc.gpsimd.memset(zerop[:], 0.0)
            nc.gpsimd.memset(yf8[:, :, :, NP1:NPAD], 0.0)
            nc.gpsimd.memset(yf8[:, :, :, 0:CLS], 0.0)
            nc.gpsimd.dma_start(hmbuf[0:1, 0:1], onep[:])
            nc.gpsimd.dma_start(hmbuf[NP1:NPAD, 0:1], zerop[0 : NPAD - NP1, 0:1])
            nc.gpsimd.dma_start(w1p[:], w1p_d[:])
            nc.gpsimd.dma_start(w1x[:], w1x_d[:])

            # ---- scalar queue: small consts ----
            nc.scalar.dma_start(b2b[:], b2b_d[:])
            nc.scalar.dma_start(b1t[:], b1t_d[:])
            nc.scalar.dma_start(hct[:], hct_d[:])

            with tc.tile_pool(name="pa2", bufs=1) as PA2:
                acT = PA2.tile([128, 4, 2, N], f8, tag="acT", name="acT")
                ybp = PA2.tile([128, 4, 2, D], f8, tag="ybp", name="ybp")

                with tc.tile_pool(name="pa1", bufs=1) as PA1:
                    yUT = PA1.tile([128, ND, N], bf, tag="yUT", name="yUT")
                    ts0 = PA1.tile([128, ND, N], bf, tag="ts0", name="ts0")
                    ts1 = PA1.tile([128, ND, N], bf, tag="ts1", name="ts1")
                    posn = PA1.tile([128, NB, N], bf, tag="posn", name="posn")

                    # ---------- phase 1: tT, yUT, pos ----------
                    with tc.tile_pool(name="p1", bufs=1) as P1, \
                         tc.tile_pool(name="ps1", bufs=2, space="PSUM") as PS1:
                        utb = P1.tile([128, ND, ND, 128], bf, tag="utb", name="utb")
                        yt = P1.tile([128, ND, NP1], bf, tag="yt", name="yt")
                        nc.sync.dma_start(utb[:], utb_d[:])
                        nc.sync.dma_start(xtb[:], xtb_d[:])
                        nc.sync.dma_start(yt[:], yt_d[:])

                        for d in range(ND):
                            ps = PS1.tile([128, N], f32, tag="psA", name="psA")
                            for k in range(ND):
                                for h in range(2):
                                    nc.tensor.matmul(
                                        ps[:, 512 * h : 512 * h + 512],
                                        utb[:, d, k],
                                        xtb[:, k, CLS + 512 * h : CLS + 512 * h + 512],
                                        start=(k == 0), stop=(k == ND - 1),
                                    )
                            nc.scalar.mul(ts0[:, d, :], ps[:], s12[:, 0, d : d + 1])
                            nc.scalar.mul(ts1[:, d, :], ps[:], s12[:, 1, d : d + 1])
                        for d in range(ND):
                            ps = PS1.tile([128, N], f32, tag="psA", name="psA")
                            for k in range(ND):
                                for h in range(2):
                                    nc.tensor.matmul(
                                        ps[:, 512 * h : 512 * h + 512],
                                        utb[:, d, k],
                                        yt[:, k, CLS + 512 * h : CLS + 512 * h + 512],
                                        start=(k == 0), stop=(k == ND - 1),
                                    )
                            nc.scalar.copy(yUT[:, d, :], ps[:])

                        # ---- pos: PE accumulate of prescaled coord planes ----
                        with tc.tile_pool(name="pcp", bufs=2) as CPP, \
                             tc.tile_pool(name="ppo", bufs=2) as PO, \
                             tc.tile_pool(name="psm0", bufs=8) as SM0, \
                             tc.tile_pool(name="psp", bufs=2, space="PSUM") as PSP:
                            for nb in range(NB):
                                cpt = CPP.tile([128, 6, N], bf, tag="cpt", name="cpt")
                                nc.sync.dma_start(cpt[:], cpl_d[nb])
                                if nb == 2:
                                    nc.sync.dma_start(ybp[:], ybp_d[:])
                                psp = PSP.tile([128, N], f32, tag="psp", name="psp")
                                for c in range(6):
                                    for h in range(2):
                                        nc.tensor.matmul(
                                            psp[:, 512 * h : 512 * h + 512],
                                            identm[:],
                                            cpt[:, c, 512 * h : 512 * h + 512],
                                            start=(c == 0), stop=(c == 5),
                                        )
                                pxp = PO.tile([128, N], bf, tag="pxp", name="pxp")
                                pss = SM0.tile([128, 1], f32, tag="pss", name="pss")
                                nc.scalar.activation(pxp[:], psp[:], AF.Exp,
                                                     bias=zerop[:],
                                                     accum_out=pss[:])
                                prg = SM0.tile([128, 1], f32, tag="prg", name="prg")
                                nc.vector.reciprocal(prg[:], pss[:])
                                nc.vector.tensor_scalar_mul(prg[:], prg[:], SA * g)
                                nc.vector.tensor_scalar_mul(
                                    posn[:, nb, :], pxp[:], prg[:])

                    # ---------- phase 2: attention, entropy, routing ----------
                    with tc.tile_pool(name="pat", bufs=4) as PT, \
                         tc.tile_pool(name="plk", bufs=2) as LK, \
                         tc.tile_pool(name="pdk", bufs=4) as DK, \
                         tc.tile_pool(name="psm", bufs=16) as SM, \
                         tc.tile_pool(name="psl", bufs=3, space="PSUM") as PSL, \
                         tc.tile_pool(name="pstp", bufs=2, space="PSUM") as PST:
                        for nb in range(NB):
                            r0 = 128 * nb
                            attn = []
                            accs = []
                            for k2 in range(2):
                                tsk = ts0 if k2 == 0 else ts1
                                psl = PSL.tile([128, N], f32, tag="psl", name="psl")
                                for e in range(ND):
                                    for h in range(2):
                                        nc.tensor.matmul(
                                            psl[:, 512 * h : 512 * h + 512],
                                            tsk[:, e, r0 : r0 + 128],
                                            yUT[:, e, 512 * h : 512 * h + 512],
                                            start=(e == 0), stop=(e == ND - 1),
                                        )
                                patch = PT.tile([128, N], bf, tag="patch", name="patch")
                                esum = SM.tile([128, 1], f32, tag="esum", name="esum")
                                nc.scalar.activation(patch[:], psl[:], AF.Exp,
                                                     bias=zerop[:], scale=SCALE,
                                                     accum_out=esum[:])
                                rk = SM.tile([128, 1], f32, tag="rk", name="rk")
                                nc.vector.reciprocal(rk[:], esum[:])
                                nc.vector.tensor_scalar_mul(rk[:], rk[:], SA * omg)
                                nc.vector.scalar_tensor_tensor(
                                    patch[:], patch[:], rk[:], posn[:, nb, :],
                                    ALU.mult, ALU.add)
                                lnk = LK.tile([128, N], bf, tag="lnk", name="lnk")
                                nc.scalar.activation(lnk[:], patch[:], AF.Ln,
                                                     bias=epsb[:])
                                # raw accum: accr = sum(attn_s * ln attn_s)
                                #          = 256*(ln256 - H)  (decreasing in H)
                                accr = SM.tile([128, 1], f32, tag="accr", name="accr")
                                ENT.tensor_mul(lnk[:], lnk[:], patch[:])
                                nc.vector.tensor_reduce(
                                    accr[:], lnk[:], axis=mybir.AxisListType.X,
                                    op=ALU.add)
                                attn.append(patch)
                                accs.append(accr)

                            # route0 iff H0<=H1 iff accr0>=accr1
                            rsel = SM.tile([128, 1], f32, tag="rsel", name="rsel")
                            nc.vector.tensor_tensor(rsel[:], accs[0][:], accs[1][:],
                                                    ALU.is_ge)
                            amax = SM.tile([128, 1], f32, tag="amax", name="amax")
                            nc.vector.tensor_tensor(amax[:], accs[0][:], accs[1][:],
                                                    ALU.max)
                            # e = exp(-ht*H_sel) = exp(ht/256*amax - ht*ln256)
                            ee = SM.tile([128, 1], f32, tag="ee", name="ee")
                            nc.scalar.activation(ee[:], amax[:], AF.Exp,
                                                 scale=ht / SA, bias=hbias[:])
                            ep1 = SM.tile([128, 1], f32, tag="ep1", name="ep1")
                            nc.vector.tensor_scalar_add(ep1[:], ee[:], 1.0)
                            rcp = SM.tile([128, 1], f32, tag="rcp", name="rcp")
                            nc.vector.reciprocal(rcp[:], ep1[:])
                            heat = SM.tile([128, 1], f32, tag="heat", name="heat")
                            nc.vector.scalar_tensor_tensor(
                                heat[:], ee[:], 2.0, rcp[:], ALU.mult, ALU.mult)
                            nc.sync.dma_start(
                                hmbuf[CLS + r0 : CLS + r0 + 128, 0:1], heat[:])
                            d01 = DK.tile([128, N], bf, tag="d01", name="d01")
                            ENT.tensor_sub(d01[:], attn[0][:], attn[1][:])
                            dka = DK.tile([128, N], f32, tag="dka", name="dka")
                            nc.vector.scalar_tensor_tensor(
                                dka[:], d01[:], rsel[:], attn[1][:],
                                ALU.mult, ALU.add)
                            for mb in range(NB):
                                pst = PST.tile([128, 128], f32, tag="pst", name="pst")
                                nc.tensor.transpose(
                                    pst[:], dka[:, 128 * mb : 128 * mb + 128],
                                    identf[:])
                                dst = acT[:, mb // 2, mb % 2, r0 : r0 + 128]
                                if mb < 4:
                                    nc.scalar.copy(dst, pst[:])
                                else:
                                    nc.vector.tensor_copy(dst, pst[:])

                # ---------- phase 3: y_outT (fp8 DoubleRow) -> yf8 ----------
                with tc.tile_pool(name="psy", bufs=2, space="PSUM") as PSY:
                    for d in range(ND):
                        psy = PSY.tile([128, N], f32, tag="psy", name="psy")
                        for mbp in range(4):
                            for h in range(2):
                                mm_dr(
                                    psy[:, 512 * h : 512 * h + 512],
                                    ybp[:, mbp, :, 128 * d : 128 * d + 128],
                                    acT[:, mbp, :, 512 * h : 512 * h + 512],
                                    (mbp == 0), (mbp == 3),
                                )
                        nc.scalar.mul(yf8[:, d // 2, d % 2, CLS : CLS + N],
                                      psy[:], SY / SA)

            # ---------- phase 4: MLP ----------
            with tc.tile_pool(name="pg", bufs=1) as PG:
                if W2BF:
                    w2r = PG.tile([128, NF, D], bf, tag="w2r", name="w2r")
                    gel = PG.tile([128, NF, NPAD], bf, tag="gel", name="gel")
                else:
                    w2r = PG.tile([128, 12, 2, D], f8, tag="w2r", name="w2r")
                    gel = PG.tile([128, 12, 2, NPAD], f8, tag="gel", name="gel")
                nc.sync.dma_start(w2r[:], w2r_d[:])

                chunksA = [(1024, NPAD - 1024), (0, 512), (512, 512)]
                with tc.tile_pool(name="psh", bufs=2, space="PSUM") as PSH:
                    for f in range(NF):
                        psh = PSH.tile([128, NPAD], f32, tag="psh", name="psh")
                        for c in range(ND):
                            for (s0, wd) in chunksA:
                                nc.tensor.matmul(
                                    psh[:, s0 : s0 + wd],
                                    w1x[:, c, f],
                                    xtb[:, c, s0 : s0 + wd],
                                    start=(c == 0), stop=False,
                                )
                        for yp in range(3):
                            for (s0, wd) in chunksA:
                                mm_dr(
                                    psh[:, s0 : s0 + wd],
                                    w1p[:, yp, f],
                                    yf8[:, yp, :, s0 : s0 + wd],
                                    False, (yp == 2),
                                )
                        # exact CLS-token y-half correction (host-computed)
                        nc.vector.tensor_add(psh[:, 0:CLS], psh[:, 0:CLS],
                                             hct[:, f : f + 1])
                        gdst = gel[:, f, :] if W2BF else gel[:, f // 2, f % 2, :]
                        nc.scalar.activation(gdst, psh[:],
                                             AF.Gelu, bias=b1t[:, f : f + 1],
                                             scale=1.0 / SW1X)

                with tc.tile_pool(name="p5", bufs=3) as P5, \
                     tc.tile_pool(name="pso", bufs=2, space="PSUM") as PSO:
                    chunksB = [(512, D - 512), (0, 512)]
                    for tb in range(NPAD // 128):
                        r0 = 128 * tb
                        nrows = min(128, NP1 - r0)
                        if nrows <= 0:
                            continue
                        xn = P5.tile([128, D], f32, tag="xn", name="xn")
                        nc.sync.dma_start(xn[:nrows, :], xnat_d[r0 : r0 + nrows, :])
                        hmc = P5.tile([128, 1], f32, tag="hmc", name="hmc")
                        nc.sync.dma_start(hmc[:nrows, :],
                                          hmbuf[r0 : r0 + nrows, 0:1])
                        pso = PSO.tile([128, D], f32, tag="pso", name="pso")
                        if W2BF:
                            for f in range(NF):
                                for (s0, wd) in chunksB:
                                    nc.tensor.matmul(
                                        pso[:, s0 : s0 + wd],
                                        gel[:, f, r0 : r0 + 128],
                                        w2r[:, f, s0 : s0 + wd],
                                        start=(f == 0), stop=(f == NF - 1),
                                    )
                        else:
                            for fp in range(12):
                                for (s0, wd) in chunksB:
                                    mm_dr(
                                        pso[:, s0 : s0 + wd],
                                        gel[:, fp, :, r0 : r0 + 128],
                                        w2r[:, fp, :, s0 : s0 + wd],
                                        (fp == 0), (fp == 11),
                                    )
                        st = P5.tile([128, D], f32, tag="st", name="st")
                        if W2BF:
                            nc.vector.tensor_add(st[:], pso[:], b2b[:])
                        else:
                            nc.vector.scalar_tensor_tensor(
                                st[:], pso[:], 1.0 / SW2, b2b[:],
                                ALU.mult, ALU.add)
                        ot = P5.tile([128, D], f32, tag="ot", name="ot")
                        nc.vector.scalar_tensor_tensor(
                            ot[:nrows, :], st[:nrows, :], hmc[:nrows, :],
                            xn[:nrows, :], ALU.mult, ALU.add)
                        nc.sync.dma_start(out_d[r0 : r0 + nrows, :], ot[:nrows, :])

    nc.compile()
    return nc


def _get_prog(g, ht, pt):
    key = (round(float(g), 9), round(float(ht), 9), round(float(pt), 9))
    if key not in _prog_cache:
        _prog_cache[key] = _build(*key)
    return _prog_cache[key]


def kernel(x, y, coords, U, S1, S2, gating, h_temp, p_temp, pos_emb, W1, b1, W2, b2):
    x = np.asarray(x, dtype=np.float32)
    y = np.asarray(y, dtype=np.float32)
    coords = np.asarray(coords, dtype=np.float32)
    U = np.asarray(U, dtype=np.float32)
    bf16 = ml_dtypes.bfloat16
    f8 = ml_dtypes.float8_e4m3

    g = float(1.0 / (1.0 + np.exp(-float(np.asarray(gating)))))
    ht = float(np.asarray(h_temp))
    pt = abs(float(np.asarray(p_temp)))
    nc = _get_prog(g, ht, pt)

    def q8(a):
        return np.clip(a, -240.0, 240.0).astype(f8)

    # ---- shared (replicated) host prep ----
    UT = np.ascontiguousarray(U.T)
    utb = np.ascontiguousarray(
        UT.reshape(ND, 128, ND, 128).transpose(1, 2, 0, 3)).astype(bf16)
    s12 = np.ascontiguousarray(np.stack(
        [np.abs(np.asarray(S1, np.float32)).reshape(ND, 128).T,
         np.abs(np.asarray(S2, np.float32)).reshape(ND, 128).T], axis=1))
    # coords planes prescaled by -|p|*pos_emb[i,c]:
    #   cpl[nb,p,c,j] = -pt*pe[128nb+p,c] * coords[128nb+p,j,c]
    pe_f = (-pt) * np.asarray(pos_emb, np.float32)[:, :, 0]   # [N, 6]
    cplw = coords.transpose(0, 2, 1) * pe_f[:, :, None]        # [N, 6, N]
    cpl = np.ascontiguousarray(
        cplw.reshape(NB, 128, 6, N)).astype(bf16)
    W1 = np.asarray(W1, np.float32)
    W1a, W1b = W1[:D], W1[D:]
    w1x = np.ascontiguousarray(
        (SW1X * W1a).reshape(ND, 128, NF, 128).transpose(1, 0, 2, 3)).astype(bf16)
    w1p = q8((SW1Y * W1b).reshape(3, 2, 128, NF, 128).transpose(2, 0, 3, 1, 4))
    W2 = np.asarray(W2, np.float32)
    if W2BF:
        w2r = np.ascontiguousarray(
            W2.reshape(NF, 128, D).transpose(1, 0, 2)).astype(bf16)
    else:
        w2r = q8((SW2 * W2).reshape(12, 2, 128, D).transpose(2, 0, 1, 3))
    b1t = np.ascontiguousarray(np.asarray(b1, np.float32).reshape(NF, 128).T)
    b2b = np.broadcast_to(np.asarray(b2, np.float32), (128, D)).copy()

    shared = {"utb": utb, "s12": s12, "cpl": cpl,
              "w1x": w1x, "w1p": w1p, "w2r": w2r, "b1t": b1t, "b2b": b2b}

    in_maps = []
    for b in range(B):
        xp = np.zeros((NPAD, D), np.float32)
        xp[:NP1] = x[b]
        xtb = np.ascontiguousarray(
            xp.T.reshape(ND, 128, NPAD).transpose(1, 0, 2)).astype(bf16)
        yt = np.ascontiguousarray(
            y[b].T.reshape(ND, 128, NP1).transpose(1, 0, 2)).astype(bf16)
        ybp = q8(y[b, CLS:].reshape(4, 2, 128, D).transpose(2, 0, 1, 3))
        xnat = np.zeros((NPAD, D), np.float32)
        xnat[:NP1] = x[b]
        # exact CLS y-half: 32*h_y[cls] = SW1X * (y_cls @ W1b)
        hc = SW1X * (y[b, 0] @ W1b)
        hct = np.ascontiguousarray(hc.reshape(NF, 128).T)
        m = dict(shared)
        m["xtb"] = xtb
        m["yt"] = yt
        m["ybp"] = ybp
        m["xnat"] = xnat
        m["hct"] = hct
        in_maps.append(m)

    res = run_bass_kernel_spmd(nc, in_maps, list(range(B)))
    out = np.stack([res.results[b]["out"][:NP1, :] for b in range(B)])
    return out.astype(np.float32)


if __name__ == "__main__":
    import time
    sys.path.insert(0, "/root/problem")
    from reference import setup_inputs, reference

    inp = {k: np.asarray(v) for k, v in setup_inputs().items()}
    t0 = time.time()
    got = kernel(**inp)
    print("kernel wall:", time.time() - t0)
    exp = np.asarray(reference(**inp))
    d = np.abs(got - exp)
    print("absmax_rel:", d.max() / np.abs(exp).max())
    print("rms_rel:", np.sqrt((d ** 2).mean()) / np.sqrt((exp ** 2).mean()))


# revision 37
# speedup vs baseline: 1.9191x; 1.1121x over previous
"""Trainium2 Bass kernel for nn_DepthCueRectification_Sp.

Data-parallel over batch: 8 batch elements -> 8 NeuronCores (SPMD).

Per-core pipeline (D=768, N=1024, token pad NPAD=1152):
  tT    = U @ xb.T                  (bf16)
  yUT   = U @ yb.T                  (bf16)   [algebra: logits_k =
                                     (|S_k|*t) @ (y@U.T).T]
  tsT_k = |S_k|-scaled copies of tT (ACT per-partition scale)
  pos   = PE identity-accumulate of host-prescaled coord planes
          (cplw[i,j,c] = -|p|*pe[i,c]*coords[i,j,c]), exp on ACT
  logits_k -> exp (no max-sub, fused row-sum) -> attn_k = 256*attn (bf16)
  entropy: Ln on ACT; multiply+reduce on Pool (raw accum); routing
  compares raw accums; heat = 2e/(1+e), e = exp(-ht*H_sel) via one exp
  dka (selected attn, f32) -> PE transpose (f32) -> acT fp8 pairs
  y_outT = fp8 DoubleRow (ybp pairs @ acT pairs) -> yf8 = 16*y_full.T fp8
  MLP W1: x-half bf16 (xtb @ 32*W1a) + y-half fp8 DR (yf8 @ 2*W1b) = 32*h
          CLS y-half zeroed on device; exact host correction hct added.
  gel   = gelu(psh/32 + b1) -> fp8 pairs
  MLP W2: fp8 DoubleRow (gel pairs @ 32*W2 pairs) = 32*xp'
  out   = x + heat*(xp' + b2)

The act-table dict is patched so Exp and Ln resolve to the combined
natural_log_exp_and_others set (avoids per-iteration table reloads).
tensor_tensor_reduce and 16-bit PE transposes hard-crash the exec unit
on this toolchain and are not used.
"""

import os
import sys

if "/opt/trn_rl_repo" not in sys.path:
    sys.path.insert(0, "/opt/trn_rl_repo")

import numpy as np
import ml_dtypes

import concourse.bass as bass
import concourse.bass_utils as _bu
import concourse.mybir as mybir
import concourse.tile as tile
from concourse import bacc
from concourse.bass_utils import run_bass_kernel_spmd
from concourse.hw_specs import get_activation_tables
from concourse.masks import make_identity

# Enable walrus's LDWEIGHTS elision (skips redundant weight reloads when
# consecutive matmuls share a stationary operand). concourse pins it off;
# correctness is covered by the rel-err check.
if int(os.environ.get("K_LDWOPT", "0")) and not getattr(_bu, "_ldwopt_patched", False):
    _orig_run_command = _bu.run_command

    def _run_command_ldwopt(cmd, **kw):
        if cmd and "walrus_driver" in str(cmd[0]):
            cmd = [c.replace("--enable-ldw-opt=false", "--enable-ldw-opt=true")
                   if isinstance(c, str) else c for c in cmd]
        return _orig_run_command(cmd, **kw)

    _bu.run_command = _run_command_ldwopt
    _bu._ldwopt_patched = True

B, N, D, DFF, CLS = 8, 1024, 768, 3072, 1
NP1 = N + CLS          # 1025
NPAD = 1152            # 9 * 128
ND = D // 128          # 6
NB = N // 128          # 8
NF = DFF // 128        # 24
AF = mybir.ActivationFunctionType
ALU = mybir.AluOpType
dt = mybir.dt
DR = mybir.MatmulPerfMode.DoubleRow

NODR = bool(int(os.environ.get("K_NODR", "0")))      # disable DoubleRow
NOPOOL = bool(int(os.environ.get("K_NOPOOL", "0")))  # entropy ops on DVE
W2BF = bool(int(os.environ.get("K_W2BF", "0")))      # W2 in bf16

SCALE = float(D) ** -0.5
SA = 256.0             # attn scale (fp8 headroom)
SY = 16.0              # y_full scale in yf8
SW1X = 32.0            # W1 x-half scale (bf16)  == SW1Y*SY
SW1Y = 2.0             # W1 y-half scale (fp8)
SW2 = 32.0             # W2 scale (fp8)
LN256 = float(np.log(SA))

_prog_cache = {}


def _patch_act_tables(arch):
    """Make natural_log_exp_and_others the only provider of Exp/Ln so the
    compiler's table-load pass keeps one table across the attention loop.
    Mutates the functools-cached dict in place (names/ids unchanged)."""
    tabs = get_activation_tables(arch)
    keep = "natural_log_exp_and_others"
    if keep not in tabs:
        return
    for name, s in tabs.items():
        if name == keep:
            continue
        s.discard(AF.Exp)
        s.discard(AF.Ln)


def _build(g, ht, pt):
    omg = 1.0 - g
    f8 = dt.float8e4
    bf = dt.bfloat16
    f32 = dt.float32

    nc = bacc.Bacc("TRN2", target_bir_lowering=False, debug=False, num_devices=8)
    _patch_act_tables(nc.m.arch)

    def mm_dr(out, l3, r3, start, stop):
        if not NODR:
            nc.tensor.matmul(out, l3, r3, start=start, stop=stop, perf_mode=DR)
        else:
            nc.tensor.matmul(out, l3[:, 0], r3[:, 0], start=start, stop=False)
            nc.tensor.matmul(out, l3[:, 1], r3[:, 1], start=False, stop=stop)

    ENT = nc.vector if NOPOOL else nc.gpsimd

    # ---- DRAM params ----
    xtb_d = nc.declare_dram_parameter("xtb", [128, ND, NPAD], bf, isOutput=False)
    yt_d = nc.declare_dram_parameter("yt", [128, ND, NP1], bf, isOutput=False)
    ybp_d = nc.declare_dram_parameter("ybp", [128, 4, 2, D], f8, isOutput=False)
    utb_d = nc.declare_dram_parameter("utb", [128, ND, ND, 128], bf, isOutput=False)
    w1p_d = nc.declare_dram_parameter("w1p", [128, 3, NF, 2, 128], f8, isOutput=False)
    w1x_d = nc.declare_dram_parameter("w1x", [128, ND, NF, 128], bf, isOutput=False)
    if W2BF:
        w2r_d = nc.declare_dram_parameter("w2r", [128, NF, D], bf, isOutput=False)
    else:
        w2r_d = nc.declare_dram_parameter("w2r", [128, 12, 2, D], f8, isOutput=False)
    b1t_d = nc.declare_dram_parameter("b1t", [128, NF], f32, isOutput=False)
    hct_d = nc.declare_dram_parameter("hct", [128, NF], f32, isOutput=False)
    b2b_d = nc.declare_dram_parameter("b2b", [128, D], f32, isOutput=False)
    s12_d = nc.declare_dram_parameter("s12", [128, 2, ND], f32, isOutput=False)
    cpl_d = nc.declare_dram_parameter("cpl", [NB, 128, 6, N], bf, isOutput=False)
    xnat_d = nc.declare_dram_parameter("xnat", [NPAD, D], f32, isOutput=False)
    out_d = nc.declare_dram_parameter("out", [NPAD, D], f32, isOutput=True)
    hmbuf = nc.dram_tensor("hmbuf", [NPAD, 1], f32)

    with tile.TileContext(nc) as tc:
        with tc.tile_pool(name="p0", bufs=1) as P0:
            # ---- persistent tiles ----
            w1p = P0.tile([128, 3, NF, 2, 128], f8, tag="w1p", name="w1p")
            w1x = P0.tile([128, ND, NF, 128], bf, tag="w1x", name="w1x")
            xtb = P0.tile([128, ND, NPAD], bf, tag="xtb", name="xtb")
            yf8 = P0.tile([128, 3, 2, NPAD], f8, tag="yf8", name="yf8")
            b2b = P0.tile([128, D], f32, tag="b2b", name="b2b")
            b1t = P0.tile([128, NF], f32, tag="b1t", name="b1t")
            hct = P0.tile([128, NF], f32, tag="hct", name="hct")
            s12 = P0.tile([128, 2, ND], f32, tag="s12", name="s12")
            identf = P0.tile([128, 128], f32, tag="identf", name="identf")
            identm = P0.tile([128, 128], bf, tag="identm", name="identm")
            epsb = P0.tile([128, 1], f32, tag="epsb", name="epsb")
            onep = P0.tile([1, 1], f32, tag="onep", name="onep")
            zerop = P0.tile([128, 1], f32, tag="zerop", name="zerop")
            hbias = P0.tile([128, 1], f32, tag="hbias", name="hbias")

            # ---- gpsimd queue: small inits, then the big weight loads ----
            nc.gpsimd.dma_start(s12[:], s12_d[:])
            make_identity(nc, identf[:])
            make_identity(nc, identm[:])
            nc.gpsimd.memset(epsb[:], SA * 1e-8)
            nc.gpsimd.memset(hbias[:], -ht * LN256)
            nc.gpsimd.memset(onep[:], 1.0)
            nc.gpsimd.memset(zerop[:], 0.0)
            nc.gpsimd.memset(yf8[:, :, :, NP1:NPAD], 0.0)
            nc.gpsimd.memset(yf8[:, :, :, 0:CLS], 0.0)
            nc.gpsimd.dma_start(hmbuf[0:1, 0:1], onep[:])
            nc.gpsimd.dma_start(hmbuf[NP1:NPAD, 0:1], zerop[0 : NPAD - NP1, 0:1])
            nc.gpsimd.dma_start(w1p[:], w1p_d[:])
            nc.gpsimd.dma_start(w1x[:], w1x_d[:])

            # ---- scalar queue: small consts ----
            nc.scalar.dma_start(b2b[:], b2b_d[:])
            nc.scalar.dma_start(b1t[:], b1t_d[:])
            nc.scalar.dma_start(hct[:], hct_d[:])

            # ---- PE warmup: ramp the tensor engine to full clock while
            # the input DMAs stream (identm has no DMA dependency). ----
            with tc.tile_pool(name="pwu", bufs=1, space="PSUM") as PWU:
                wps = PWU.tile([128, 128], f32, tag="wps", name="wps")
                for _ in range(24):
                    nc.tensor.matmul(wps[:], identm[:], identm[:],
                                     start=True, stop=True)

            with tc.tile_pool(name="pa2", bufs=1) as PA2:
                acT = PA2.tile([128, 4, 2, N], f8, tag="acT", name="acT")
                ybp = PA2.tile([128, 4, 2, D], f8, tag="ybp", name="ybp")

                with tc.tile_pool(name="pa1", bufs=1) as PA1:
                    yUT = PA1.tile([128, ND, N], bf, tag="yUT", name="yUT")
                    ts0 = PA1.tile([128, ND, N], bf, tag="ts0", name="ts0")
                    ts1 = PA1.tile([128, ND, N], bf, tag="ts1", name="ts1")
                    posn = PA1.tile([128, NB, N], bf, tag="posn", name="posn")

                    # ---------- phase 1: tT, yUT, pos ----------
                    with tc.tile_pool(name="p1", bufs=1) as P1, \
                         tc.tile_pool(name="ps1", bufs=2, space="PSUM") as PS1:
                        utb = P1.tile([128, ND, ND, 128], bf, tag="utb", name="utb")
                        yt = P1.tile([128, ND, NP1], bf, tag="yt", name="yt")
                        nc.scalar.dma_start(utb[:], utb_d[:])
                        nc.sync.dma_start(xtb[:], xtb_d[:])
                        nc.scalar.dma_start(yt[:], yt_d[:])

                        for d in range(ND):
                            ps = PS1.tile([128, N], f32, tag="psA", name="psA")
                            for k in range(ND):
                                for h in range(2):
                                    nc.tensor.matmul(
                                        ps[:, 512 * h : 512 * h + 512],
                                        utb[:, d, k],
                                        xtb[:, k, CLS + 512 * h : CLS + 512 * h + 512],
                                        start=(k == 0), stop=(k == ND - 1),
                                    )
                            nc.scalar.mul(ts0[:, d, :], ps[:], s12[:, 0, d : d + 1])
                            nc.scalar.mul(ts1[:, d, :], ps[:], s12[:, 1, d : d + 1])
                        for d in range(ND):
                            ps = PS1.tile([128, N], f32, tag="psA", name="psA")
                            for k in range(ND):
                                for h in range(2):
                                    nc.tensor.matmul(
                                        ps[:, 512 * h : 512 * h + 512],
                                        utb[:, d, k],
                                        yt[:, k, CLS + 512 * h : CLS + 512 * h + 512],
                                        start=(k == 0), stop=(k == ND - 1),
                                    )
                            nc.scalar.copy(yUT[:, d, :], ps[:])

                    # ---- phase 2: pos (PE-accumulated) interleaved with
                    #      attention, entropy, routing ----
                    with tc.tile_pool(name="pcp", bufs=2) as CPP, \
                         tc.tile_pool(name="ppo", bufs=2) as PO, \
                         tc.tile_pool(name="psm0", bufs=4) as SM0, \
                         tc.tile_pool(name="pat", bufs=3) as PT, \
                         tc.tile_pool(name="plk", bufs=2) as LK, \
                         tc.tile_pool(name="pdk", bufs=2) as DK, \
                         tc.tile_pool(name="psm", bufs=8) as SM, \
                         tc.tile_pool(name="psp", bufs=1, space="PSUM") as PSP, \
                         tc.tile_pool(name="psl", bufs=2, space="PSUM") as PSL, \
                         tc.tile_pool(name="pstp", bufs=2, space="PSUM") as PST:

                        def emit_pos(nb):
                            cpt = CPP.tile([128, 6, N], bf, tag="cpt", name="cpt")
                            nc.sync.dma_start(cpt[:], cpl_d[nb])
                            psp = PSP.tile([128, N], f32, tag="psp", name="psp")
                            for c in range(6):
                                for h in range(2):
                                    nc.tensor.matmul(
                                        psp[:, 512 * h : 512 * h + 512],
                                        identm[:],
                                        cpt[:, c, 512 * h : 512 * h + 512],
                                        start=(c == 0), stop=(c == 5),
                                    )
                            pxp = PO.tile([128, N], bf, tag="pxp", name="pxp")
                            pss = SM0.tile([128, 1], f32, tag="pss", name="pss")
                            nc.scalar.activation(pxp[:], psp[:], AF.Exp,
                                                 bias=zerop[:], accum_out=pss[:])
                            prg = SM0.tile([128, 1], f32, tag="prg", name="prg")
                            nc.vector.reciprocal(prg[:], pss[:])
                            nc.vector.tensor_scalar_mul(prg[:], prg[:], SA * g)
                            nc.vector.tensor_scalar_mul(
                                posn[:, nb, :], pxp[:], prg[:])

                        emit_pos(0)
                        emit_pos(1)
                        for nb in range(NB):
                            if nb == 2:
                                nc.gpsimd.dma_start(ybp[:], ybp_d[:])
                            r0 = 128 * nb
                            pk = PT.tile([128, 2, N], bf, tag="pk", name="pk")
                            lnk = LK.tile([128, 2, N], bf, tag="lnk", name="lnk")
                            accr = SM.tile([128, 2], f32, tag="accr", name="accr")
                            for k2 in range(2):
                                tsk = ts0 if k2 == 0 else ts1
                                psl = PSL.tile([128, N], f32, tag="psl", name="psl")
                                for e in range(ND):
                                    for h in range(2):
                                        nc.tensor.matmul(
                                            psl[:, 512 * h : 512 * h + 512],
                                            tsk[:, e, r0 : r0 + 128],
                                            yUT[:, e, 512 * h : 512 * h + 512],
                                            start=(e == 0), stop=(e == ND - 1),
                                        )
                                patch = pk[:, k2, :]
                                esum = SM.tile([128, 1], f32, tag="esum", name="esum")
                                nc.scalar.activation(patch, psl[:], AF.Exp,
                                                     bias=zerop[:], scale=SCALE,
                                                     accum_out=esum[:])
                                rk = SM.tile([128, 1], f32, tag="rk", name="rk")
                                nc.vector.reciprocal(rk[:], esum[:])
                                nc.vector.tensor_scalar_mul(rk[:], rk[:], SA * omg)
                                nc.vector.scalar_tensor_tensor(
                                    patch, patch, rk[:], posn[:, nb, :],
                                    ALU.mult, ALU.add)
                                nc.scalar.activation(lnk[:, k2, :], patch, AF.Ln,
                                                     bias=epsb[:])
                            if 2 + nb < NB:
                                emit_pos(2 + nb)
                            # raw accum: accr_k = sum(attn_s * ln attn_s)
                            #          = 256*(ln256 - H_k)  (decreasing in H)
                            ENT.tensor_mul(lnk[:], lnk[:], pk[:])
                            nc.vector.tensor_reduce(
                                accr[:], lnk[:], axis=mybir.AxisListType.X,
                                op=ALU.add)
                            # route0 iff H0<=H1 iff accr0>=accr1
                            rsel = SM.tile([128, 1], f32, tag="rsel", name="rsel")
                            nc.vector.tensor_tensor(rsel[:], accr[:, 0:1],
                                                    accr[:, 1:2], ALU.is_ge)
                            amax = SM.tile([128, 1], f32, tag="amax", name="amax")
                            nc.vector.tensor_tensor(amax[:], accr[:, 0:1],
                                                    accr[:, 1:2], ALU.max)
                            # e = exp(-ht*H_sel) = exp(ht/256*amax - ht*ln256)
                            ee = SM.tile([128, 1], f32, tag="ee", name="ee")
                            nc.scalar.activation(ee[:], amax[:], AF.Exp,
                                                 scale=ht / SA, bias=hbias[:])
                            ep1 = SM.tile([128, 1], f32, tag="ep1", name="ep1")
                            nc.vector.tensor_scalar_add(ep1[:], ee[:], 1.0)
                            rcp = SM.tile([128, 1], f32, tag="rcp", name="rcp")
                            nc.vector.reciprocal(rcp[:], ep1[:])
                            heat = SM.tile([128, 1], f32, tag="heat", name="heat")
                            nc.vector.scalar_tensor_tensor(
                                heat[:], ee[:], 2.0, rcp[:], ALU.mult, ALU.mult)
                            nc.sync.dma_start(
                                hmbuf[CLS + r0 : CLS + r0 + 128, 0:1], heat[:])
                            d01 = DK.tile([128, N], bf, tag="d01", name="d01")
                            nc.vector.tensor_sub(d01[:], pk[:, 0, :], pk[:, 1, :])
                            dka = DK.tile([128, N], f32, tag="dka", name="dka")
                            nc.vector.scalar_tensor_tensor(
                                dka[:], d01[:], rsel[:], pk[:, 1, :],
                                ALU.mult, ALU.add)
                            for mb in range(NB):
                                pst = PST.tile([128, 128], f32, tag="pst", name="pst")
                                nc.tensor.transpose(
                                    pst[:], dka[:, 128 * mb : 128 * mb + 128],
                                    identf[:])
                                dst = acT[:, mb // 2, mb % 2, r0 : r0 + 128]
                                nc.scalar.copy(dst, pst[:])

                # ---------- phase 3: y_outT (fp8 DoubleRow) -> yf8 ----------
                with tc.tile_pool(name="psy", bufs=2, space="PSUM") as PSY:
                    for d in range(ND):
                        psy = PSY.tile([128, N], f32, tag="psy", name="psy")
                        for mbp in range(4):
                            for h in range(2):
                                mm_dr(
                                    psy[:, 512 * h : 512 * h + 512],
                                    ybp[:, mbp, :, 128 * d : 128 * d + 128],
                                    acT[:, mbp, :, 512 * h : 512 * h + 512],
                                    (mbp == 0), (mbp == 3),
                                )
                        nc.scalar.mul(yf8[:, d // 2, d % 2, CLS : CLS + N],
                                      psy[:], SY / SA)

            # ---------- phase 4: MLP ----------
            with tc.tile_pool(name="pg", bufs=1) as PG:
                if W2BF:
                    w2r = PG.tile([128, NF, D], bf, tag="w2r", name="w2r")
                    gel = PG.tile([128, NF, NPAD], bf, tag="gel", name="gel")
                else:
                    w2r = PG.tile([128, 12, 2, D], f8, tag="w2r", name="w2r")
                    gel = PG.tile([128, 12, 2, NPAD], f8, tag="gel", name="gel")
                nc.sync.dma_start(w2r[:], w2r_d[:])
                # pad token columns of gel are never computed; zero once so
                # the tb=8 W2 stationary reads are NaN-free
                if W2BF:
                    nc.gpsimd.memset(gel[:, :, NP1:NPAD], 0.0)
                else:
                    nc.gpsimd.memset(gel[:, :, :, NP1:NPAD], 0.0)

                # tokens 0..1023 in two 512 chunks; token 1024 (last) alone
                chunksA = [(1024, 1), (0, 512), (512, 512)]
                with tc.tile_pool(name="psh", bufs=2, space="PSUM") as PSH:
                    for f in range(NF):
                        psh = PSH.tile([128, NPAD], f32, tag="psh", name="psh")
                        for c in range(ND):
                            for (s0, wd) in chunksA:
                                nc.tensor.matmul(
                                    psh[:, s0 : s0 + wd],
                                    w1x[:, c, f],
                                    xtb[:, c, s0 : s0 + wd],
                                    start=(c == 0), stop=False,
                                )
                        for yp in range(3):
                            for (s0, wd) in chunksA:
                                mm_dr(
                                    psh[:, s0 : s0 + wd],
                                    w1p[:, yp, f],
                                    yf8[:, yp, :, s0 : s0 + wd],
                                    False, (yp == 2),
                                )
                        # exact CLS-token y-half correction (host-computed)
                        nc.vector.tensor_add(psh[:, 0:CLS], psh[:, 0:CLS],
                                             hct[:, f : f + 1])
                        gdst = (gel[:, f, 0:NP1] if W2BF
                                else gel[:, f // 2, f % 2, 0:NP1])
                        nc.scalar.activation(gdst, psh[:, 0:NP1],
                                             AF.Gelu, bias=b1t[:, f : f + 1],
                                             scale=1.0 / SW1X)

                with tc.tile_pool(name="p5", bufs=3) as P5, \
                     tc.tile_pool(name="pso", bufs=2, space="PSUM") as PSO:
                    chunksB = [(512, D - 512), (0, 512)]
                    for tb in range(NPAD // 128):
                        r0 = 128 * tb
                        nrows = min(128, NP1 - r0)
                        if nrows <= 0:
                            continue
                        xn = P5.tile([128, D], f32, tag="xn", name="xn")
                        nc.sync.dma_start(xn[:nrows, :], xnat_d[r0 : r0 + nrows, :])
                        hmc = P5.tile([128, 1], f32, tag="hmc", name="hmc")
                        nc.sync.dma_start(hmc[:nrows, :],
                                          hmbuf[r0 : r0 + nrows, 0:1])
                        pso = PSO.tile([128, D], f32, tag="pso", name="pso")
                        if W2BF:
                            for f in range(NF):
                                for (s0, wd) in chunksB:
                                    nc.tensor.matmul(
                                        pso[:, s0 : s0 + wd],
                                        gel[:, f, r0 : r0 + 128],
                                        w2r[:, f, s0 : s0 + wd],
                                        start=(f == 0), stop=(f == NF - 1),
                                    )
                        else:
                            for fp in range(12):
                                for (s0, wd) in chunksB:
                                    mm_dr(
                                        pso[:, s0 : s0 + wd],
                                        gel[:, fp, :, r0 : r0 + 128],
                                        w2r[:, fp, :, s0 : s0 + wd],
                                        (fp == 0), (fp == 11),
                                    )
                        st = P5.tile([128, D], f32, tag="st", name="st")
                        if W2BF:
                            nc.vector.tensor_add(st[:], pso[:], b2b[:])
                        else:
                            nc.vector.scalar_tensor_tensor(
                                st[:], pso[:], 1.0 / SW2, b2b[:],
                                ALU.mult, ALU.add)
                        ot = P5.tile([128, D], f32, tag="ot", name="ot")
                        nc.vector.scalar_tensor_tensor(
                            ot[:nrows, :], st[:nrows, :], hmc[:nrows, :],
                            xn[:nrows, :], ALU.mult, ALU.add)
                        nc.sync.dma_start(out_d[r0 : r0 + nrows, :], ot[:nrows, :])

    nc.compile()
    return nc


def _get_prog(g, ht, pt):
    key = (round(float(g), 9), round(float(ht), 9), round(float(pt), 9))
    if key not in _prog_cache:
        _prog_cache[key] = _build(*key)
    return _prog_cache[key]


def kernel(x, y, coords, U, S1, S2, gating, h_temp, p_temp, pos_emb, W1, b1, W2, b2):
    x = np.asarray(x, dtype=np.float32)
    y = np.asarray(y, dtype=np.float32)
    coords = np.asarray(coords, dtype=np.float32)
    U = np.asarray(U, dtype=np.float32)
    bf16 = ml_dtypes.bfloat16
    f8 = ml_dtypes.float8_e4m3

    g = float(1.0 / (1.0 + np.exp(-float(np.asarray(gating)))))
    ht = float(np.asarray(h_temp))
    pt = abs(float(np.asarray(p_temp)))
    nc = _get_prog(g, ht, pt)

    def q8(a):
        return np.clip(a, -240.0, 240.0).astype(f8)

    # ---- shared (replicated) host prep ----
    UT = np.ascontiguousarray(U.T)
    utb = np.ascontiguousarray(
        UT.reshape(ND, 128, ND, 128).transpose(1, 2, 0, 3)).astype(bf16)
    s12 = np.ascontiguousarray(np.stack(
        [np.abs(np.asarray(S1, np.float32)).reshape(ND, 128).T,
         np.abs(np.asarray(S2, np.float32)).reshape(ND, 128).T], axis=1))
    # coords planes prescaled by -|p|*pos_emb[i,c]:
    #   cpl[nb,p,c,j] = -pt*pe[128nb+p,c] * coords[128nb+p,j,c]
    pe_f = (-pt) * np.asarray(pos_emb, np.float32)[:, :, 0]   # [N, 6]
    cplw = coords.transpose(0, 2, 1) * pe_f[:, :, None]        # [N, 6, N]
    cpl = np.ascontiguousarray(
        cplw.reshape(NB, 128, 6, N)).astype(bf16)
    W1 = np.asarray(W1, np.float32)
    W1a, W1b = W1[:D], W1[D:]
    w1x = np.ascontiguousarray(
        (SW1X * W1a).reshape(ND, 128, NF, 128).transpose(1, 0, 2, 3)).astype(bf16)
    w1p = q8((SW1Y * W1b).reshape(3, 2, 128, NF, 128).transpose(2, 0, 3, 1, 4))
    W2 = np.asarray(W2, np.float32)
    if W2BF:
        w2r = np.ascontiguousarray(
            W2.reshape(NF, 128, D).transpose(1, 0, 2)).astype(bf16)
    else:
        w2r = q8((SW2 * W2).reshape(12, 2, 128, D).transpose(2, 0, 1, 3))
    b1t = np.ascontiguousarray(np.asarray(b1, np.float32).reshape(NF, 128).T)
    b2b = np.broadcast_to(np.asarray(b2, np.float32), (128, D)).copy()

    shared = {"utb": utb, "s12": s12, "cpl": cpl,
              "w1x": w1x, "w1p": w1p, "w2r": w2r, "b1t": b1t, "b2b": b2b}

    in_maps = []
    for b in range(B):
        xp = np.zeros((NPAD, D), np.float32)
        xp[:NP1] = x[b]
        xtb = np.ascontiguousarray(
            xp.T.reshape(ND, 128, NPAD).transpose(1, 0, 2)).astype(bf16)
        yt = np.ascontiguousarray(
            y[b].T.reshape(ND, 128, NP1).transpose(1, 0, 2)).astype(bf16)
        ybp = q8(y[b, CLS:].reshape(4, 2, 128, D).transpose(2, 0, 1, 3))
        xnat = np.zeros((NPAD, D), np.float32)
        xnat[:NP1] = x[b]
        # exact CLS y-half: 32*h_y[cls] = SW1X * (y_cls @ W1b)
        hc = SW1X * (y[b, 0] @ W1b)
        hct = np.ascontiguousarray(hc.reshape(NF, 128).T)
        m = dict(shared)
        m["xtb"] = xtb
        m["yt"] = yt
        m["ybp"] = ybp
        m["xnat"] = xnat
        m["hct"] = hct
        in_maps.append(m)

    res = run_bass_kernel_spmd(nc, in_maps, list(range(B)))
    out = np.stack([res.results[b]["out"][:NP1, :] for b in range(B)])
    return out.astype(np.float32)


if __name__ == "__main__":
    import time
    sys.path.insert(0, "/root/problem")
    from reference import setup_inputs, reference

    inp = {k: np.asarray(v) for k, v in setup_inputs().items()}
    t0 = time.time()
    got = kernel(**inp)
    print("kernel wall:", time.time() - t0)
    exp = np.asarray(reference(**inp))
    d = np.abs(got - exp)
    print("absmax_rel:", d.max() / np.abs(exp).max())
    print("rms_rel:", np.sqrt((d ** 2).mean()) / np.sqrt((exp ** 2).mean()))
